# revision 30
# baseline (speedup 1.0000x reference)
"""Distributed multi-head attention kernel for one TRN2 chip (8 NeuronCores).

Problem: B=2, T=2048, D=1024, H=16 heads (hd=64).
  qkv = x @ w_attn + b_attn ; per-head softmax((q k^T)/sqrt(hd) + 2*mask) v
  out = attn @ w_proj + b_proj

Sharding: tensor-parallel over heads. Core c owns heads {2c, 2c+1}.
  - Q/K projections in transposed layout (QT/KT: [hd, T]); head h lives at
    partitions [64h, 64h+64) of qt2/kt2.
  - S^T[kv, q] = K_h^T Q_h as K=64 row-tiled matmul pairs: the two heads
    occupy PE row-halves (tile_position (0,0)/(64,0)) and stream
    concurrently - 2x over the zero-padded K=128 formulation.
  - V is computed directly in natural [t, hd] layout (lhsT = x chunk,
    rhs = w_v), eliminating the PE transposes of the old design.
  - Softmax denominator comes from a ones-column appended to V in the PV
    matmul (O_ext = [V|1]^T @ P^T); PV is stream-bound and runs at the
    N-cycle roofline already.
  - Exp runs on ScalarE only (it is the pacing engine); all PSUM->SBUF
    copies are on VectorE.
  - Mid-kernel AllToAll (two halves, one per batch) reshards from
    head-space to row-space; the final projection then needs no reduction.
  - All matmul operands bf16; accumulation + softmax statistics fp32.
"""

import sys

sys.path.insert(0, "/opt/trn_rl_repo")

import numpy as np

B, T, D = 2, 2048, 1024
H = 16
HD = D // H
NCORES = 8
HPC = H // NCORES          # heads per core = 2
BT = B * T                 # 4096 global rows
ROWS_PER_CORE = BT // NCORES   # 512
RHALF = ROWS_PER_CORE // 2     # 256 rows per A2A half
TB = 512                   # t-block width for QKV projection
NTB = BT // TB             # 8
NKD = D // 128             # 8 contraction chunks over D
QB = 512                   # q-block width in attention
NQB = T // QB              # 4 per (batch, head)
NKV = T // 128             # 16 kv chunks per batch
PVLAG = 6                  # PV trails exp by this many kv chunks

_CACHE = {}
import ml_dtypes
BF16 = ml_dtypes.bfloat16


def _build(with_mask: bool, with_battn: bool, with_bproj: bool):
    import concourse.bass as bass
    import concourse.tile as tile
    from concourse import bacc, mybir

    f32 = mybir.dt.float32
    bf16 = mybir.dt.bfloat16
    Exp = mybir.ActivationFunctionType.Exp

    nc = bacc.Bacc("TRN2", target_bir_lowering=False, debug=False,
                   num_devices=NCORES)
    rg = [list(range(NCORES))]

    xt = nc.dram_tensor("xt", [D, BT], bf16, kind="ExternalInput")
    w_qk = nc.dram_tensor("w_qk", [D, 256], bf16, kind="ExternalInput")
    w_v = nc.dram_tensor("w_v", [D, 128], bf16, kind="ExternalInput")
    w_proj = nc.dram_tensor("w_proj", [D, D], bf16, kind="ExternalInput")
    if with_mask:
        mask2 = nc.dram_tensor("mask2", [128, B * NKV], f32,
                               kind="ExternalInput")
    if with_battn:
        b_qk = nc.dram_tensor("b_qk", [1, 256], bf16, kind="ExternalInput")
        b_v = nc.dram_tensor("b_v", [1, 128], bf16, kind="ExternalInput")
    if with_bproj:
        b_proj = nc.dram_tensor("b_proj", [1, D], bf16, kind="ExternalInput")
    out = nc.dram_tensor("out", [ROWS_PER_CORE, D], f32, kind="ExternalOutput")

    with tile.TileContext(nc, num_cores=NCORES) as tc:
        from contextlib import ExitStack
        with ExitStack() as ctx:
            const = ctx.enter_context(tc.tile_pool(name="const", bufs=1))
            xt_pool = ctx.enter_context(tc.tile_pool(name="xtp", bufs=4))
            qk_pool = ctx.enter_context(tc.tile_pool(name="qkp", bufs=1))
            pt_pool = ctx.enter_context(tc.tile_pool(name="ptp", bufs=9))
            lbc_pool = ctx.enter_context(tc.tile_pool(name="lbc", bufs=2))
            pin_pool = ctx.enter_context(tc.tile_pool(name="pin", bufs=8))
            out_pool = ctx.enter_context(tc.tile_pool(name="outp", bufs=2))
            # [128, 1024] f32 = 2 PSUM banks per slot; 3 slots = 6 banks.
            # One slot holds BOTH heads' S^T chunk (h0 -> bank a, h1 ->
            # bank b), so the row-tiled matmul pair shares one slot
            # dependency and issues back-to-back (concurrent row groups),
            # and exp runs as a single N=1024 ACTIVATE.
            hot = ctx.enter_context(tc.tile_pool(name="hot", bufs=3,
                                                 space="PSUM"))
            acc_pool = ctx.enter_context(tc.tile_pool(name="accp", bufs=2,
                                                      space="PSUM"))
            dram = ctx.enter_context(tc.tile_pool(name="dram", bufs=2,
                                                  space="DRAM"))

            # ---- ACT exp-table preload (runs at t~0 on the scalar queue) --
            dmy = const.tile([1, 8], bf16)
            nc.vector.memset(dmy[:], 0.0)
            dmy2 = const.tile([1, 8], f32)
            nc.scalar.activation(out=dmy2[:], in_=dmy[:], func=Exp)

            # ---- constants ----
            # w_qk packed per D-chunk: [128, NKD, 256]; group g cols
            # [128g, 128g+128): g0=[q_h0/8|k_h0] g1=[q_h1/8|k_h1]
            wqk_sb = const.tile([128, NKD, 256], bf16)
            for half in range(2):
                nc.sync.dma_start(
                    out=wqk_sb[:, 4 * half:4 * (half + 1), :],
                    in_=w_qk[:].rearrange("(a p) c -> p a c", p=128)[
                        :, 4 * half:4 * (half + 1), :])
            # w_v packed per D-chunk: [128, NKD, 128]; cols [v_h0|v_h1]
            wv_sb = const.tile([128, NKD, 128], bf16)
            nc.scalar.dma_start(
                out=wv_sb[:],
                in_=w_v[:].rearrange("(a p) c -> p a c", p=128))
            wproj_sb = const.tile([128, NKD, D], bf16)

            def emit_wproj_loads():
                for half in range(2):
                    nc.sync.dma_start(
                        out=wproj_sb[:, 4 * half:4 * (half + 1), :],
                        in_=w_proj[:].rearrange("(a p) c -> p a c", p=128)[
                            :, 4 * half:4 * (half + 1), :])

            if with_mask:
                # mask (already doubled on host): [128, B, NKV]
                mask_sb = const.tile([128, B, NKV], f32)
                nc.sync.dma_start(out=mask_sb[:],
                                  in_=mask2[:].rearrange("p (b j) -> p b j",
                                                         b=B))
            if with_battn:
                bqk_sb = const.tile([1, 256], bf16)
                nc.sync.dma_start(out=bqk_sb[:], in_=b_qk[:])
                bv_sb = const.tile([1, 128], bf16)
                nc.sync.dma_start(out=bv_sb[:], in_=b_v[:])
                ones_row = const.tile([1, TB], bf16)
                nc.vector.memset(ones_row[:], 1.0)
            if with_bproj:
                bproj_sb = const.tile([1, D], bf16)
                nc.sync.dma_start(out=bproj_sb[:], in_=b_proj[:])
                ones_col = const.tile([1, 128], bf16)
                nc.vector.memset(ones_col[:], 1.0)

            ones64 = const.tile([1, 64], bf16)
            nc.vector.memset(ones64[:], 1.0)

            # persistent activations. Head h at partitions [64h, 64h+64).
            qt2 = qk_pool.tile([128, BT], bf16, tag="qt2", name="qt2")
            kt2 = qk_pool.tile([128, BT], bf16, tag="kt2", name="kt2")
            ot = qk_pool.tile([128, BT], bf16, tag="ot", name="ot")
            # V natural layout + ones column: subtile s = 32b + 2j + h is
            # [128 kv, 65] = [V_chunk | 1].
            vnat = qk_pool.tile([128, 2 * NKV * HPC, 65], bf16, tag="vnat",
                                name="vnat")
            nc.vector.memset(vnat[:, :, 64:65], 1.0)

            # ---- QKV projection ----
            # q/k: transposed outputs via stationary weights.
            # v: natural output via stationary x chunks.
            def qkv_tblock_units(tb):
                xt_t = xt_pool.tile([128, NKD, TB], bf16, tag="xt", name="xt")

                def dma_unit():
                    xsrc = xt[:].rearrange("(a p) t -> p a t", p=128)
                    for half in range(2):
                        nc.sync.dma_start(
                            out=xt_t[:, 4 * half:4 * (half + 1), :],
                            in_=xsrc[:, 4 * half:4 * (half + 1),
                                     TB * tb:TB * (tb + 1)])

                def qk_unit(g):
                    ps = hot.tile([128, 2 * TB], f32, tag="hot", name="qkps")
                    for d in range(NKD):
                        nc.tensor.matmul(
                            ps[:, 0:TB],
                            lhsT=wqk_sb[:, d, 128 * g:128 * (g + 1)],
                            rhs=xt_t[:, d, :],
                            start=(d == 0),
                            stop=(d == NKD - 1) and not with_battn)
                    if with_battn:
                        nc.tensor.matmul(
                            ps[:, 0:TB],
                            lhsT=bqk_sb[:, 128 * g:128 * (g + 1)],
                            rhs=ones_row[:],
                            start=False, stop=True)
                    cs = TB * tb
                    nc.vector.tensor_copy(
                        out=qt2[64 * g:64 * (g + 1), cs:cs + TB],
                        in_=ps[0:64, 0:TB])
                    nc.vector.tensor_copy(
                        out=kt2[64 * g:64 * (g + 1), cs:cs + TB],
                        in_=ps[64:128, 0:TB])

                def v_unit(vh):
                    # two 128-row t-chunks per unit
                    ps = hot.tile([128, 2 * TB], f32, tag="hot", name="vps")
                    for ch in range(2):
                        c4 = 2 * vh + ch
                        for d in range(NKD):
                            nc.tensor.matmul(
                                ps[:, 128 * ch:128 * (ch + 1)],
                                lhsT=xt_t[:, d, 128 * c4:128 * (c4 + 1)],
                                rhs=wv_sb[:, d, :],
                                start=(d == 0),
                                stop=(d == NKD - 1) and not with_battn)
                        if with_battn:
                            nc.tensor.matmul(
                                ps[:, 128 * ch:128 * (ch + 1)],
                                lhsT=ones_row[:, 0:128],
                                rhs=bv_sb[:],
                                start=False, stop=True)
                    for ch in range(2):
                        c4 = 2 * vh + ch
                        gj = 4 * tb + c4          # global 128-row chunk id
                        b_, j = gj // NKV, gj % NKV
                        for h in range(HPC):
                            s = 32 * b_ + 2 * j + h
                            nc.vector.tensor_copy(
                                out=vnat[:, s, 0:64],
                                in_=ps[:, 128 * ch + 64 * h:
                                       128 * ch + 64 * (h + 1)])

                # finer-grained halves so PE-queue fillers never stall
                # the scalar engine for more than ~1us
                qkps = {}

                def qk_half(g, half):
                    if g not in qkps:
                        qkps[g] = hot.tile([128, 2 * TB], f32, tag="hot",
                                           name="qkps")
                    ps = qkps[g]
                    for d in range(4 * half, 4 * half + 4):
                        nc.tensor.matmul(
                            ps[:, 0:TB],
                            lhsT=wqk_sb[:, d, 128 * g:128 * (g + 1)],
                            rhs=xt_t[:, d, :],
                            start=(d == 0),
                            stop=(d == NKD - 1) and not with_battn)
                    if half == 0:
                        return
                    if with_battn:
                        nc.tensor.matmul(
                            ps[:, 0:TB],
                            lhsT=bqk_sb[:, 128 * g:128 * (g + 1)],
                            rhs=ones_row[:],
                            start=False, stop=True)
                    cs = TB * tb
                    nc.vector.tensor_copy(
                        out=qt2[64 * g:64 * (g + 1), cs:cs + TB],
                        in_=ps[0:64, 0:TB])
                    nc.vector.tensor_copy(
                        out=kt2[64 * g:64 * (g + 1), cs:cs + TB],
                        in_=ps[64:128, 0:TB])
                    del qkps[g]

                def v_chunk(c4):
                    ps = hot.tile([128, 2 * TB], f32, tag="hot", name="vps")
                    for d in range(NKD):
                        nc.tensor.matmul(
                            ps[:, 0:128],
                            lhsT=xt_t[:, d, 128 * c4:128 * (c4 + 1)],
                            rhs=wv_sb[:, d, :],
                            start=(d == 0),
                            stop=(d == NKD - 1) and not with_battn)
                    if with_battn:
                        nc.tensor.matmul(
                            ps[:, 0:128],
                            lhsT=ones_row[:, 0:128],
                            rhs=bv_sb[:],
                            start=False, stop=True)
                    gj = 4 * tb + c4          # global 128-row chunk id
                    b_, j = gj // NKV, gj % NKV
                    for h in range(HPC):
                        s = 32 * b_ + 2 * j + h
                        nc.vector.tensor_copy(
                            out=vnat[:, s, 0:64],
                            in_=ps[:, 64 * h:64 * (h + 1)])

                return [dma_unit,
                        lambda: qk_half(0, 0), lambda: qk_half(0, 1),
                        lambda: qk_half(1, 0), lambda: qk_half(1, 1),
                        lambda: v_chunk(0), lambda: v_chunk(1),
                        lambda: v_chunk(2), lambda: v_chunk(3)]

            # ---- attention: one global software-pipelined chunk stream ----
            # chunk g = (b, qb, j); S^T+exp at position g, PV at g+PVLAG.
            # The PV lag spans q-block boundaries so the normalize of
            # q-block Q overlaps the S^T/exp flow of Q+1.
            CHUNKS = [(b, qb, j) for b in range(B) for qb in range(NQB)
                      for j in range(NKV)]
            pts = {}
            ps_o_by_q = {}

            def emit_st(g):
                b, qb, j = CHUNKS[g]
                c0 = 2048 * b + QB * qb
                k0 = 2048 * b + 128 * j
                # both heads' S^T chunk in one 2-bank slot; the two K=64
                # row-tiled matmuls hit disjoint PE row groups and
                # disjoint banks -> concurrent
                st = hot.tile([128, 2 * QB], f32, tag="hot", name="st")
                for h in range(HPC):
                    nc.tensor.matmul(
                        st[:, QB * h:QB * (h + 1)],
                        lhsT=kt2[64 * h:64 * (h + 1), k0:k0 + 128],
                        rhs=qt2[64 * h:64 * (h + 1), c0:c0 + QB],
                        start=True, stop=True)
                pt = pt_pool.tile([128, 2 * QB], bf16, tag="pt", name="pt")
                if with_mask:
                    nc.scalar.activation(out=pt[:], in_=st[:], func=Exp,
                                         bias=mask_sb[:, b, j:j + 1],
                                         scale=1.0)
                else:
                    nc.scalar.activation(out=pt[:], in_=st[:], func=Exp)
                pts[g] = pt

            def emit_pv(g):
                b, qb, j = CHUNKS[g]
                if j == 0:
                    ps_o_by_q[(b, qb)] = [
                        acc_pool.tile([65, QB], f32, tag="acc", name="acc")
                        for _ in range(HPC)]
                ps_o = ps_o_by_q[(b, qb)]
                pt = pts.pop(g)
                for h in range(HPC):
                    s = 32 * b + 2 * j + h
                    nc.tensor.matmul(
                        ps_o[h][:],
                        lhsT=vnat[:, s, :],
                        rhs=pt[:, QB * h:QB * (h + 1)],
                        start=(j == 0), stop=(j == NKV - 1),
                        skip_group_check=True)
                if j == NKV - 1:
                    normalize(b, qb)

            def normalize(b, qb):
                # drain ps_o to SBUF with two proven-shape copies (64 and
                # 1 partitions) so the 2 accumulator banks recycle fast;
                # recip + DRAM broadcast roundtrip + mul run off the
                # critical path on SBUF data
                c0 = 2048 * b + QB * qb
                ps_o = ps_o_by_q.pop((b, qb))
                for h in range(HPC):
                    osum = lbc_pool.tile([64, QB], f32, tag="osum",
                                         name="osum")
                    nc.vector.tensor_copy(out=osum[:], in_=ps_o[h][0:64, :])
                    lsb = lbc_pool.tile([1, QB], f32, tag="lsb", name="lsb")
                    nc.vector.tensor_copy(out=lsb[:], in_=ps_o[h][64:65, :])
                    lrec = lbc_pool.tile([1, QB], f32, tag="lrec",
                                         name="lrec")
                    nc.vector.reciprocal_approx_fast(
                        out=lrec[:], in_=lsb[:])
                    ldram = dram.tile([1, QB], f32, tag="ld", name="ld")
                    nc.sync.dma_start(out=ldram[:], in_=lrec[:])
                    lbc = lbc_pool.tile([64, QB], f32, tag="lbc", name="lbc")
                    nc.sync.dma_start(out=lbc[:],
                                      in_=ldram[:].to_broadcast([64, QB]))
                    nc.vector.tensor_mul(
                        out=ot[64 * h:64 * (h + 1), c0:c0 + QB],
                        in0=osum[:], in1=lbc[:])
                a2a_stage(b, qb)

            # ---- A2A half (reshard head-space -> row-space) ----
            # each batch-half reshards via TWO half-width collectives
            # (t-cols 0:128 / 128:256 of every destination chunk); the
            # first one alone unblocks proj rb=0, pipelining the final
            # projection with the second collective's delivery
            a2a_outs = {}
            a2a_ins = {}
            RH2 = RHALF // 2

            def a2a_stage(half, qb):
                if half not in a2a_ins:
                    a2a_ins[half] = [
                        dram.tile([NCORES, 128, RH2], bf16, tag=f"a2ain{s}",
                                  name="a2ain")
                        for s in range(2)]
                for j in (2 * qb, 2 * qb + 1):
                    c0 = 2048 * half + RHALF * j
                    for s in range(2):
                        nc.sync.dma_start(
                            out=a2a_ins[half][s][j],
                            in_=ot[:, c0 + RH2 * s:c0 + RH2 * (s + 1)])

            def a2a_half(half):
                from concourse import mybir as _mb
                outs = []
                for s in range(2):
                    a_out = dram.tile([NCORES, 128, RH2], bf16,
                                      tag=f"a2aout{s}", name="a2aout")
                    nc.gpsimd.collective_compute(
                        "AllToAll", _mb.AluOpType.bypass, replica_groups=rg,
                        ins=[a2a_ins[half][s].opt()], outs=[a_out.opt()])
                    outs.append(a_out)
                a2a_outs[half] = outs

            pins_by_half = {}

            def pins_load(half, s=None):
                # on the gpsimd queue so the wait-for-collective does not
                # head-of-line-block the sync queue's normalize DMAs
                if half not in pins_by_half:
                    pins_by_half[half] = [
                        pin_pool.tile([128, RHALF], bf16, tag="pin",
                                      name="pin")
                        for _ in range(NCORES)]
                pins = pins_by_half[half]
                for ss in ((0, 1) if s is None else (s,)):
                    for j in range(NCORES):
                        nc.gpsimd.dma_start(
                            out=pins[j][:, RH2 * ss:RH2 * (ss + 1)],
                            in_=a2a_outs[half][ss][j])

            proj_state = {}

            def proj_unit(half, rb, n):
                pins = pins_by_half[half]
                if (half, rb) not in proj_state:
                    proj_state[(half, rb)] = (
                        out_pool.tile([128, D], f32, tag="osb", name="osb"),
                        hot.tile([128, 1024], f32, tag="hot", name="pjps"))
                osb, ps = proj_state[(half, rb)]
                for j in range(NCORES):
                    nc.tensor.matmul(
                        ps[:, 512 * n:512 * (n + 1)],
                        lhsT=pins[j][:, 128 * rb:128 * (rb + 1)],
                        rhs=wproj_sb[:, j, 512 * n:512 * (n + 1)],
                        start=(j == 0),
                        stop=(j == NCORES - 1) and not with_bproj)
                if with_bproj:
                    nc.tensor.matmul(
                        ps[:, 512 * n:512 * (n + 1)], lhsT=ones_col[:],
                        rhs=bproj_sb[:, 512 * n:512 * (n + 1)],
                        start=False, stop=True)
                nc.vector.tensor_copy(out=osb[:, 512 * n:512 * (n + 1)],
                                      in_=ps[:, 512 * n:512 * (n + 1)])
                if n == 1:
                    del proj_state[(half, rb)]
                    r0 = 256 * half + 128 * rb
                    nc.sync.dma_start(out=out[r0:r0 + 128, :], in_=osb[:])

            def proj_rb(half, rb):
                proj_unit(half, rb, 0)
                proj_unit(half, rb, 1)

            # ---- emission order ----
            # u[tb] = [dma, qk0a, qk0b, qk1a, qk1b, v0, v1, v2, v3]
            u = {tb: qkv_tblock_units(tb) for tb in range(NTB)}
            for f in u[0][:5]:    # tb0 dma+qk only; v deferred so the
                f()               # first S^T isn't queued behind it
            fill = {}

            def put(g, *fns):
                fill.setdefault(g, []).extend(fns)

            # b0 phase: just-in-time qk for tb1-3 (kt chunk 4j needed by
            # chunk 4j), V units placed PVLAG chunks before their PV
            put(0, u[0][5])
            put(1, u[1][0], u[1][1])
            put(2, u[1][2], u[1][3])
            put(3, u[1][4], u[0][6])
            put(4, u[0][7], u[0][8])
            put(5, u[2][0], u[1][5])
            put(6, u[2][1], u[2][2])
            put(7, u[2][3], u[2][4])
            put(8, u[1][6], u[1][7])
            put(9, u[3][0], u[1][8])
            put(10, u[3][1], u[3][2])
            put(11, u[3][3], u[3][4])
            put(12, u[2][5], u[2][6])
            put(13, u[2][7], u[2][8])
            put(14, u[3][5], u[3][6])
            put(15, u[3][7], u[3][8])
            # b1 QKV: no urgency, one unit per chunk
            for i, tb in enumerate((4, 5, 6, 7)):
                g0 = 18 + 10 * i
                for k in range(9):
                    put(g0 + k, u[tb][k])
            put(57, emit_wproj_loads)
            put(76, lambda: a2a_half(0))
            put(82, lambda: pins_load(0))

            NG = len(CHUNKS)
            for g in range(NG + PVLAG):
                if g in fill:
                    for fn in fill[g]:
                        fn()
                if g < NG:
                    emit_st(g)
                if g >= PVLAG:
                    emit_pv(g - PVLAG)
            # final reshard + projection; proj(0) runs inside the second
            # collective's window, with dummy matmuls sprinkled in to keep
            # the PE's HAM clock warm until proj(1)'s inputs arrive
            def dummy_mms(n):
                ps = hot.tile([128, 1024], f32, tag="hot", name="warm")
                for i in range(n):
                    nc.tensor.matmul(
                        ps[:, 512 * (i % 2):512 * (i % 2 + 1)],
                        lhsT=wproj_sb[:, 0, 0:128],
                        rhs=wproj_sb[:, 1, 0:512],
                        start=True, stop=True)

            a2a_half(1)
            dummy_mms(4)
            proj_unit(0, 0, 0)
            dummy_mms(2)
            proj_unit(0, 0, 1)
            dummy_mms(2)
            proj_unit(0, 1, 0)
            dummy_mms(2)
            proj_unit(0, 1, 1)
            dummy_mms(8)
            pins_load(1, 0)
            proj_rb(1, 0)       # needs only pins cols 0:128 (collective a)
            pins_load(1, 1)
            proj_rb(1, 1)

    nc.finalize()
    return nc


def _prep_inputs(x, attention_mask, w_attn, b_attn, w_proj, b_proj):
    x = np.asarray(x, np.float32)
    xt = np.ascontiguousarray(x.reshape(BT, D).T).astype(BF16)
    w_attn = np.asarray(w_attn, np.float32)
    b_attn = np.asarray(b_attn, np.float32)
    wp = np.ascontiguousarray(np.asarray(w_proj, np.float32)).astype(BF16)
    scale = 1.0 / np.sqrt(HD)
    am = np.asarray(attention_mask, np.float32)
    with_mask = bool(np.any(am))
    with_battn = bool(np.any(b_attn))
    with_bproj = bool(np.any(np.asarray(b_proj)))
    mask2 = None
    if with_mask:
        m2 = (2.0 * am).reshape(B, T // 128, 128)
        mask2 = np.ascontiguousarray(m2.transpose(2, 0, 1).reshape(128, -1))
    in_maps = []
    for c in range(NCORES):
        h0, h1 = HPC * c, HPC * c + 1
        qkcols = []
        vcols = []
        for h in (h0, h1):
            qkcols.append(w_attn[:, HD * h:HD * (h + 1)] * scale)      # q
            qkcols.append(w_attn[:, D + HD * h:D + HD * (h + 1)])      # k
            vcols.append(w_attn[:, 2 * D + HD * h:2 * D + HD * (h + 1)])
        wqk = np.ascontiguousarray(np.concatenate(qkcols, axis=1)).astype(BF16)
        wv = np.ascontiguousarray(np.concatenate(vcols, axis=1)).astype(BF16)
        m = {"xt": xt, "w_qk": wqk, "w_v": wv, "w_proj": wp}
        if with_mask:
            m["mask2"] = mask2
        if with_battn:
            bqk = []
            bv = []
            for h in (h0, h1):
                bqk.append(b_attn[HD * h:HD * (h + 1)] * scale)
                bqk.append(b_attn[D + HD * h:D + HD * (h + 1)])
                bv.append(b_attn[2 * D + HD * h:2 * D + HD * (h + 1)])
            m["b_qk"] = np.ascontiguousarray(
                np.concatenate(bqk)[None, :].astype(BF16))
            m["b_v"] = np.ascontiguousarray(
                np.concatenate(bv)[None, :].astype(BF16))
        if with_bproj:
            m["b_proj"] = np.ascontiguousarray(
                np.asarray(b_proj, np.float32)[None, :].astype(BF16))
        in_maps.append(m)
    return in_maps, (with_mask, with_battn, with_bproj)


def _run(inputs, trace=False, tmpdir=None):
    from concourse.bass_utils import run_bass_kernel_spmd

    in_maps, key = _prep_inputs(**inputs)
    if key not in _CACHE:
        _CACHE[key] = _build(*key)
    nc = _CACHE[key]
    try:
        res = run_bass_kernel_spmd(nc, in_maps, core_ids=list(range(NCORES)),
                                   trace=trace, tmpdir=tmpdir)
    except Exception as e:
        if "unrecoverable" not in str(e) and "UNAVAILABLE" not in str(e):
            raise
        import ctypes
        lib = ctypes.CDLL("/opt/axon/libaxon_pjrt.so")
        if hasattr(lib, "axon_reset"):
            lib.axon_reset.restype = ctypes.c_int64
            lib.axon_reset()
        res = run_bass_kernel_spmd(nc, in_maps, core_ids=list(range(NCORES)),
                                   trace=trace, tmpdir=tmpdir)
    y = np.empty((B, T, D), np.float32)
    for c in range(NCORES):
        o = res.results[c]["out"]
        y[0, RHALF * c:RHALF * (c + 1)] = o[:RHALF]
        y[1, RHALF * c:RHALF * (c + 1)] = o[RHALF:]
    return y, res


def kernel(**inputs) -> np.ndarray:
    y, _ = _run(inputs, trace=False)
    return y


# revision 31
# speedup vs baseline: 1.2535x; 1.2535x over previous
"""Distributed multi-head attention kernel for one TRN2 chip (8 NeuronCores).

Problem: B=2, T=2048, D=1024, H=16 heads (hd=64).
  qkv = x @ w_attn + b_attn ; per-head softmax((q k^T)/sqrt(hd) + 2*mask) v
  out = attn @ w_proj + b_proj

Sharding: tensor-parallel over heads. Core c owns heads {2c, 2c+1}.
  - Q/K projections in transposed layout (QT/KT: [hd, T]); head h lives at
    partitions [64h, 64h+64) of qt2/kt2.
  - S^T[kv, q] = K_h^T Q_h as K=64 row-tiled matmul pairs: the two heads
    occupy PE row-halves (tile_position (0,0)/(64,0)) and stream
    concurrently - 2x over the zero-padded K=128 formulation.
  - V is computed directly in natural [t, hd] layout (lhsT = x chunk,
    rhs = w_v), eliminating the PE transposes of the old design.
  - Softmax denominator comes from a ones-column appended to V in the PV
    matmul (O_ext = [V|1]^T @ P^T); PV is stream-bound and runs at the
    N-cycle roofline already.
  - Exp runs on ScalarE only (it is the pacing engine); all PSUM->SBUF
    copies are on VectorE.
  - Mid-kernel AllToAll (two halves, one per batch) reshards from
    head-space to row-space; the final projection then needs no reduction.
  - All matmul operands bf16; accumulation + softmax statistics fp32.
"""

import sys

sys.path.insert(0, "/opt/trn_rl_repo")

import numpy as np

B, T, D = 2, 2048, 1024
H = 16
HD = D // H
NCORES = 8
HPC = H // NCORES          # heads per core = 2
BT = B * T                 # 4096 global rows
ROWS_PER_CORE = BT // NCORES   # 512
RHALF = ROWS_PER_CORE // 2     # 256 rows per A2A half
TB = 512                   # t-block width for QKV projection
NTB = BT // TB             # 8
NKD = D // 128             # 8 contraction chunks over D
QB = 512                   # q-block width in attention
NQB = T // QB              # 4 per (batch, head)
NKV = T // 128             # 16 kv chunks per batch
PVLAG = 6                  # PV trails exp by this many kv chunks

_CACHE = {}
import ml_dtypes
BF16 = ml_dtypes.bfloat16


def _build(with_mask: bool, with_battn: bool, with_bproj: bool):
    import concourse.bass as bass
    import concourse.tile as tile
    from concourse import bacc, mybir

    f32 = mybir.dt.float32
    bf16 = mybir.dt.bfloat16
    Exp = mybir.ActivationFunctionType.Exp

    nc = bacc.Bacc("TRN2", target_bir_lowering=False, debug=False,
                   num_devices=NCORES)
    rg = [list(range(NCORES))]

    xt = nc.dram_tensor("xt", [D, BT], bf16, kind="ExternalInput")
    w_qk = nc.dram_tensor("w_qk", [D, 256], bf16, kind="ExternalInput")
    w_v = nc.dram_tensor("w_v", [D, 128], bf16, kind="ExternalInput")
    w_proj = nc.dram_tensor("w_proj", [D, D], bf16, kind="ExternalInput")
    if with_mask:
        mask2 = nc.dram_tensor("mask2", [128, B * NKV], f32,
                               kind="ExternalInput")
    if with_battn:
        b_qk = nc.dram_tensor("b_qk", [1, 256], bf16, kind="ExternalInput")
        b_v = nc.dram_tensor("b_v", [1, 128], bf16, kind="ExternalInput")
    if with_bproj:
        b_proj = nc.dram_tensor("b_proj", [1, D], bf16, kind="ExternalInput")
    out = nc.dram_tensor("out", [ROWS_PER_CORE, D], f32, kind="ExternalOutput")

    with tile.TileContext(nc, num_cores=NCORES) as tc:
        from contextlib import ExitStack
        with ExitStack() as ctx:
            const = ctx.enter_context(tc.tile_pool(name="const", bufs=1))
            xt_pool = ctx.enter_context(tc.tile_pool(name="xtp", bufs=4))
            qk_pool = ctx.enter_context(tc.tile_pool(name="qkp", bufs=1))
            pt_pool = ctx.enter_context(tc.tile_pool(name="ptp", bufs=9))
            lbc_pool = ctx.enter_context(tc.tile_pool(name="lbc", bufs=2))
            pin_pool = ctx.enter_context(tc.tile_pool(name="pin", bufs=8))
            out_pool = ctx.enter_context(tc.tile_pool(name="outp", bufs=2))
            # [128, 1024] f32 = 2 PSUM banks per slot; 3 slots = 6 banks.
            # One slot holds BOTH heads' S^T chunk (h0 -> bank a, h1 ->
            # bank b), so the row-tiled matmul pair shares one slot
            # dependency and issues back-to-back (concurrent row groups),
            # and exp runs as a single N=1024 ACTIVATE.
            hot = ctx.enter_context(tc.tile_pool(name="hot", bufs=3,
                                                 space="PSUM"))
            acc_pool = ctx.enter_context(tc.tile_pool(name="accp", bufs=2,
                                                      space="PSUM"))
            dram = ctx.enter_context(tc.tile_pool(name="dram", bufs=2,
                                                  space="DRAM"))

            # ---- ACT exp-table preload (runs at t~0 on the scalar queue) --
            dmy = const.tile([1, 8], bf16)
            nc.vector.memset(dmy[:], 0.0)
            dmy2 = const.tile([1, 8], f32)
            nc.scalar.activation(out=dmy2[:], in_=dmy[:], func=Exp)

            # ---- constants ----
            # w_qk packed per D-chunk: [128, NKD, 256]; group g cols
            # [128g, 128g+128): g0=[q_h0/8|k_h0] g1=[q_h1/8|k_h1]
            wqk_sb = const.tile([128, NKD, 256], bf16)
            for half in range(2):
                nc.sync.dma_start(
                    out=wqk_sb[:, 4 * half:4 * (half + 1), :],
                    in_=w_qk[:].rearrange("(a p) c -> p a c", p=128)[
                        :, 4 * half:4 * (half + 1), :])
            # w_v packed per D-chunk: [128, NKD, 128]; cols [v_h0|v_h1]
            wv_sb = const.tile([128, NKD, 128], bf16)
            nc.scalar.dma_start(
                out=wv_sb[:],
                in_=w_v[:].rearrange("(a p) c -> p a c", p=128))
            wproj_sb = const.tile([128, NKD, D], bf16)

            def emit_wproj_loads():
                for half in range(2):
                    nc.sync.dma_start(
                        out=wproj_sb[:, 4 * half:4 * (half + 1), :],
                        in_=w_proj[:].rearrange("(a p) c -> p a c", p=128)[
                            :, 4 * half:4 * (half + 1), :])

            if with_mask:
                # mask (already doubled on host): [128, B, NKV]
                mask_sb = const.tile([128, B, NKV], f32)
                nc.sync.dma_start(out=mask_sb[:],
                                  in_=mask2[:].rearrange("p (b j) -> p b j",
                                                         b=B))
            if with_battn:
                bqk_sb = const.tile([1, 256], bf16)
                nc.sync.dma_start(out=bqk_sb[:], in_=b_qk[:])
                bv_sb = const.tile([1, 128], bf16)
                nc.sync.dma_start(out=bv_sb[:], in_=b_v[:])
                ones_row = const.tile([1, TB], bf16)
                nc.vector.memset(ones_row[:], 1.0)
            if with_bproj:
                bproj_sb = const.tile([1, D], bf16)
                nc.sync.dma_start(out=bproj_sb[:], in_=b_proj[:])
                ones_col = const.tile([1, 128], bf16)
                nc.vector.memset(ones_col[:], 1.0)

            ones64 = const.tile([1, 64], bf16)
            nc.vector.memset(ones64[:], 1.0)

            # persistent activations. Head h at partitions [64h, 64h+64).
            qt2 = qk_pool.tile([128, BT], bf16, tag="qt2", name="qt2")
            kt2 = qk_pool.tile([128, BT], bf16, tag="kt2", name="kt2")
            ot = qk_pool.tile([128, BT], bf16, tag="ot", name="ot")
            # V natural layout + ones column: subtile s = 32b + 2j + h is
            # [128 kv, 65] = [V_chunk | 1].
            vnat = qk_pool.tile([128, 2 * NKV * HPC, 65], bf16, tag="vnat",
                                name="vnat")
            nc.vector.memset(vnat[:, :, 64:65], 1.0)

            # ---- QKV projection ----
            # q/k: transposed outputs via stationary weights.
            # v: natural output via stationary x chunks.
            def qkv_tblock_units(tb):
                xt_t = xt_pool.tile([128, NKD, TB], bf16, tag="xt", name="xt")

                def dma_unit():
                    xsrc = xt[:].rearrange("(a p) t -> p a t", p=128)
                    for half in range(2):
                        nc.sync.dma_start(
                            out=xt_t[:, 4 * half:4 * (half + 1), :],
                            in_=xsrc[:, 4 * half:4 * (half + 1),
                                     TB * tb:TB * (tb + 1)])

                def qk_unit(g):
                    ps = hot.tile([128, 2 * TB], f32, tag="hot", name="qkps")
                    for d in range(NKD):
                        nc.tensor.matmul(
                            ps[:, 0:TB],
                            lhsT=wqk_sb[:, d, 128 * g:128 * (g + 1)],
                            rhs=xt_t[:, d, :],
                            start=(d == 0),
                            stop=(d == NKD - 1) and not with_battn)
                    if with_battn:
                        nc.tensor.matmul(
                            ps[:, 0:TB],
                            lhsT=bqk_sb[:, 128 * g:128 * (g + 1)],
                            rhs=ones_row[:],
                            start=False, stop=True)
                    cs = TB * tb
                    nc.vector.tensor_copy(
                        out=qt2[64 * g:64 * (g + 1), cs:cs + TB],
                        in_=ps[0:64, 0:TB])
                    nc.vector.tensor_copy(
                        out=kt2[64 * g:64 * (g + 1), cs:cs + TB],
                        in_=ps[64:128, 0:TB])

                def v_unit(vh):
                    # two 128-row t-chunks per unit
                    ps = hot.tile([128, 2 * TB], f32, tag="hot", name="vps")
                    for ch in range(2):
                        c4 = 2 * vh + ch
                        for d in range(NKD):
                            nc.tensor.matmul(
                                ps[:, 128 * ch:128 * (ch + 1)],
                                lhsT=xt_t[:, d, 128 * c4:128 * (c4 + 1)],
                                rhs=wv_sb[:, d, :],
                                start=(d == 0),
                                stop=(d == NKD - 1) and not with_battn)
                        if with_battn:
                            nc.tensor.matmul(
                                ps[:, 128 * ch:128 * (ch + 1)],
                                lhsT=ones_row[:, 0:128],
                                rhs=bv_sb[:],
                                start=False, stop=True)
                    for ch in range(2):
                        c4 = 2 * vh + ch
                        gj = 4 * tb + c4          # global 128-row chunk id
                        b_, j = gj // NKV, gj % NKV
                        for h in range(HPC):
                            s = 32 * b_ + 2 * j + h
                            nc.vector.tensor_copy(
                                out=vnat[:, s, 0:64],
                                in_=ps[:, 128 * ch + 64 * h:
                                       128 * ch + 64 * (h + 1)])

                # finer-grained halves so PE-queue fillers never stall
                # the scalar engine for more than ~1us
                qkps = {}

                def qk_half(g, half):
                    if g not in qkps:
                        qkps[g] = hot.tile([128, 2 * TB], f32, tag="hot",
                                           name="qkps")
                    ps = qkps[g]
                    for d in range(4 * half, 4 * half + 4):
                        nc.tensor.matmul(
                            ps[:, 0:TB],
                            lhsT=wqk_sb[:, d, 128 * g:128 * (g + 1)],
                            rhs=xt_t[:, d, :],
                            start=(d == 0),
                            stop=(d == NKD - 1) and not with_battn)
                    if half == 0:
                        return
                    if with_battn:
                        nc.tensor.matmul(
                            ps[:, 0:TB],
                            lhsT=bqk_sb[:, 128 * g:128 * (g + 1)],
                            rhs=ones_row[:],
                            start=False, stop=True)
                    cs = TB * tb
                    nc.vector.tensor_copy(
                        out=qt2[64 * g:64 * (g + 1), cs:cs + TB],
                        in_=ps[0:64, 0:TB])
                    nc.vector.tensor_copy(
                        out=kt2[64 * g:64 * (g + 1), cs:cs + TB],
                        in_=ps[64:128, 0:TB])
                    del qkps[g]

                def v_chunk(c4):
                    ps = hot.tile([128, 2 * TB], f32, tag="hot", name="vps")
                    for d in range(NKD):
                        nc.tensor.matmul(
                            ps[:, 0:128],
                            lhsT=xt_t[:, d, 128 * c4:128 * (c4 + 1)],
                            rhs=wv_sb[:, d, :],
                            start=(d == 0),
                            stop=(d == NKD - 1) and not with_battn)
                    if with_battn:
                        nc.tensor.matmul(
                            ps[:, 0:128],
                            lhsT=ones_row[:, 0:128],
                            rhs=bv_sb[:],
                            start=False, stop=True)
                    gj = 4 * tb + c4          # global 128-row chunk id
                    b_, j = gj // NKV, gj % NKV
                    for h in range(HPC):
                        s = 32 * b_ + 2 * j + h
                        nc.vector.tensor_copy(
                            out=vnat[:, s, 0:64],
                            in_=ps[:, 64 * h:64 * (h + 1)])

                return [dma_unit,
                        lambda: qk_half(0, 0), lambda: qk_half(0, 1),
                        lambda: qk_half(1, 0), lambda: qk_half(1, 1),
                        lambda: v_chunk(0), lambda: v_chunk(1),
                        lambda: v_chunk(2), lambda: v_chunk(3)]

            # ---- attention: one global software-pipelined chunk stream ----
            # chunk g = (b, qb, j); S^T+exp at position g, PV at g+PVLAG.
            # The PV lag spans q-block boundaries so the normalize of
            # q-block Q overlaps the S^T/exp flow of Q+1.
            CHUNKS = [(b, qb, j) for b in range(B) for qb in range(NQB)
                      for j in range(NKV)]
            pts = {}
            ps_o_by_q = {}

            def emit_st(g):
                b, qb, j = CHUNKS[g]
                c0 = 2048 * b + QB * qb
                k0 = 2048 * b + 128 * j
                # both heads' S^T chunk in one 2-bank slot; the two K=64
                # row-tiled matmuls hit disjoint PE row groups and
                # disjoint banks -> concurrent
                st = hot.tile([128, 2 * QB], f32, tag="hot", name="st")
                for h in range(HPC):
                    nc.tensor.matmul(
                        st[:, QB * h:QB * (h + 1)],
                        lhsT=kt2[64 * h:64 * (h + 1), k0:k0 + 128],
                        rhs=qt2[64 * h:64 * (h + 1), c0:c0 + QB],
                        start=True, stop=True)
                pt = pt_pool.tile([128, 2 * QB], bf16, tag="pt", name="pt")
                if with_mask:
                    nc.scalar.activation(out=pt[:], in_=st[:], func=Exp,
                                         bias=mask_sb[:, b, j:j + 1],
                                         scale=1.0)
                else:
                    nc.scalar.activation(out=pt[:], in_=st[:], func=Exp)
                pts[g] = pt

            def emit_pv(g):
                b, qb, j = CHUNKS[g]
                if j == 0:
                    ps_o_by_q[(b, qb)] = [
                        acc_pool.tile([65, QB], f32, tag="acc", name="acc")
                        for _ in range(HPC)]
                ps_o = ps_o_by_q[(b, qb)]
                pt = pts.pop(g)
                for h in range(HPC):
                    s = 32 * b + 2 * j + h
                    nc.tensor.matmul(
                        ps_o[h][:],
                        lhsT=vnat[:, s, :],
                        rhs=pt[:, QB * h:QB * (h + 1)],
                        start=(j == 0), stop=(j == NKV - 1),
                        skip_group_check=True)
                if j == NKV - 1:
                    normalize(b, qb)

            def normalize(b, qb):
                # drain ps_o to SBUF with two proven-shape copies (64 and
                # 1 partitions) so the 2 accumulator banks recycle fast;
                # recip + DRAM broadcast roundtrip + mul run off the
                # critical path on SBUF data
                c0 = 2048 * b + QB * qb
                ps_o = ps_o_by_q.pop((b, qb))
                for h in range(HPC):
                    osum = lbc_pool.tile([64, QB], f32, tag="osum",
                                         name="osum")
                    nc.vector.tensor_copy(out=osum[:], in_=ps_o[h][0:64, :])
                    lsb = lbc_pool.tile([1, QB], f32, tag="lsb", name="lsb")
                    nc.vector.tensor_copy(out=lsb[:], in_=ps_o[h][64:65, :])
                    lrec = lbc_pool.tile([1, QB], f32, tag="lrec",
                                         name="lrec")
                    nc.vector.reciprocal_approx_fast(
                        out=lrec[:], in_=lsb[:])
                    ldram = dram.tile([1, QB], f32, tag="ld", name="ld")
                    nc.sync.dma_start(out=ldram[:], in_=lrec[:])
                    lbc = lbc_pool.tile([64, QB], f32, tag="lbc", name="lbc")
                    nc.sync.dma_start(out=lbc[:],
                                      in_=ldram[:].to_broadcast([64, QB]))
                    nc.vector.tensor_mul(
                        out=ot[64 * h:64 * (h + 1), c0:c0 + QB],
                        in0=osum[:], in1=lbc[:])
                a2a_stage(b, qb)

            # ---- A2A half (reshard head-space -> row-space) ----
            a2a_outs = {}
            a2a_ins = {}

            def a2a_stage(half, qb):
                if half not in a2a_ins:
                    a2a_ins[half] = dram.tile([NCORES, 128, RHALF], bf16,
                                              tag="a2ain", name="a2ain")
                a_in = a2a_ins[half]
                for j in (2 * qb, 2 * qb + 1):
                    nc.sync.dma_start(
                        out=a_in[j],
                        in_=ot[:, 2048 * half + RHALF * j:
                               2048 * half + RHALF * (j + 1)])

            def a2a_half(half):
                from concourse import mybir as _mb
                a_in = a2a_ins[half]
                a_out = dram.tile([NCORES, 128, RHALF], bf16, tag="a2aout",
                                  name="a2aout")
                nc.gpsimd.collective_compute(
                    "AllToAll", _mb.AluOpType.bypass, replica_groups=rg,
                    ins=[a_in.opt()], outs=[a_out.opt()])
                a2a_outs[half] = a_out

            pins_by_half = {}

            def pins_load(half):
                # on the gpsimd queue so the wait-for-collective does not
                # head-of-line-block the sync queue's normalize DMAs
                a_out = a2a_outs[half]
                pins = []
                for j in range(NCORES):
                    p = pin_pool.tile([128, RHALF], bf16, tag="pin",
                                      name="pin")
                    nc.gpsimd.dma_start(out=p[:], in_=a_out[j])
                    pins.append(p)
                pins_by_half[half] = pins

            proj_state = {}

            def proj_unit(half, rb, n):
                pins = pins_by_half[half]
                if (half, rb) not in proj_state:
                    proj_state[(half, rb)] = (
                        out_pool.tile([128, D], f32, tag="osb", name="osb"),
                        hot.tile([128, 1024], f32, tag="hot", name="pjps"))
                osb, ps = proj_state[(half, rb)]
                for j in range(NCORES):
                    nc.tensor.matmul(
                        ps[:, 512 * n:512 * (n + 1)],
                        lhsT=pins[j][:, 128 * rb:128 * (rb + 1)],
                        rhs=wproj_sb[:, j, 512 * n:512 * (n + 1)],
                        start=(j == 0),
                        stop=(j == NCORES - 1) and not with_bproj)
                if with_bproj:
                    nc.tensor.matmul(
                        ps[:, 512 * n:512 * (n + 1)], lhsT=ones_col[:],
                        rhs=bproj_sb[:, 512 * n:512 * (n + 1)],
                        start=False, stop=True)
                nc.vector.tensor_copy(out=osb[:, 512 * n:512 * (n + 1)],
                                      in_=ps[:, 512 * n:512 * (n + 1)])
                if n == 1:
                    del proj_state[(half, rb)]
                    r0 = 256 * half + 128 * rb
                    nc.sync.dma_start(out=out[r0:r0 + 128, :], in_=osb[:])

            def proj_rb(half, rb):
                proj_unit(half, rb, 0)
                proj_unit(half, rb, 1)

            # ---- emission order ----
            # u[tb] = [dma, qk0a, qk0b, qk1a, qk1b, v0, v1, v2, v3]
            u = {tb: qkv_tblock_units(tb) for tb in range(NTB)}
            for f in u[0][:5]:    # tb0 dma+qk only; v deferred so the
                f()               # first S^T isn't queued behind it
            fill = {}

            def put(g, *fns):
                fill.setdefault(g, []).extend(fns)

            # b0 phase: just-in-time qk for tb1-3 (kt chunk 4j needed by
            # chunk 4j), V units placed PVLAG chunks before their PV
            put(0, u[0][5])
            put(1, u[1][0], u[1][1])
            put(2, u[1][2], u[1][3])
            put(3, u[1][4], u[0][6])
            put(4, u[0][7], u[0][8])
            put(5, u[2][0], u[1][5])
            put(6, u[2][1], u[2][2])
            put(7, u[2][3], u[2][4])
            put(8, u[1][6], u[1][7])
            put(9, u[3][0], u[1][8])
            put(10, u[3][1], u[3][2])
            put(11, u[3][3], u[3][4])
            put(12, u[2][5], u[2][6])
            put(13, u[2][7], u[2][8])
            put(14, u[3][5], u[3][6])
            put(15, u[3][7], u[3][8])
            # b1 QKV: no urgency, one unit per chunk
            for i, tb in enumerate((4, 5, 6, 7)):
                g0 = 18 + 10 * i
                for k in range(9):
                    put(g0 + k, u[tb][k])
            put(57, emit_wproj_loads)
            put(76, lambda: a2a_half(0))
            put(82, lambda: pins_load(0))

            NG = len(CHUNKS)
            for g in range(NG + PVLAG):
                if g in fill:
                    for fn in fill[g]:
                        fn()
                if g < NG:
                    emit_st(g)
                if g >= PVLAG:
                    emit_pv(g - PVLAG)
            # final reshard + projection; proj(0) runs inside the second
            # collective's window, with dummy matmuls sprinkled in to keep
            # the PE's HAM clock warm until proj(1)'s inputs arrive
            def dummy_mms(n):
                ps = hot.tile([128, 1024], f32, tag="hot", name="warm")
                for i in range(n):
                    nc.tensor.matmul(
                        ps[:, 512 * (i % 2):512 * (i % 2 + 1)],
                        lhsT=wproj_sb[:, 0, 0:128],
                        rhs=wproj_sb[:, 1, 0:512],
                        start=True, stop=True)

            a2a_half(1)
            dummy_mms(4)
            proj_unit(0, 0, 0)
            dummy_mms(2)
            proj_unit(0, 0, 1)
            dummy_mms(2)
            proj_unit(0, 1, 0)
            dummy_mms(2)
            proj_unit(0, 1, 1)
            dummy_mms(12)
            pins_load(1)
            proj_rb(1, 0)
            proj_rb(1, 1)

    nc.finalize()
    return nc


def _prep_inputs(x, attention_mask, w_attn, b_attn, w_proj, b_proj):
    x = np.asarray(x, np.float32)
    xt = np.ascontiguousarray(x.reshape(BT, D).T).astype(BF16)
    w_attn = np.asarray(w_attn, np.float32)
    b_attn = np.asarray(b_attn, np.float32)
    wp = np.ascontiguousarray(np.asarray(w_proj, np.float32)).astype(BF16)
    scale = 1.0 / np.sqrt(HD)
    am = np.asarray(attention_mask, np.float32)
    with_mask = bool(np.any(am))
    with_battn = bool(np.any(b_attn))
    with_bproj = bool(np.any(np.asarray(b_proj)))
    mask2 = None
    if with_mask:
        m2 = (2.0 * am).reshape(B, T // 128, 128)
        mask2 = np.ascontiguousarray(m2.transpose(2, 0, 1).reshape(128, -1))
    in_maps = []
    for c in range(NCORES):
        h0, h1 = HPC * c, HPC * c + 1
        qkcols = []
        vcols = []
        for h in (h0, h1):
            qkcols.append(w_attn[:, HD * h:HD * (h + 1)] * scale)      # q
            qkcols.append(w_attn[:, D + HD * h:D + HD * (h + 1)])      # k
            vcols.append(w_attn[:, 2 * D + HD * h:2 * D + HD * (h + 1)])
        wqk = np.ascontiguousarray(np.concatenate(qkcols, axis=1)).astype(BF16)
        wv = np.ascontiguousarray(np.concatenate(vcols, axis=1)).astype(BF16)
        m = {"xt": xt, "w_qk": wqk, "w_v": wv, "w_proj": wp}
        if with_mask:
            m["mask2"] = mask2
        if with_battn:
            bqk = []
            bv = []
            for h in (h0, h1):
                bqk.append(b_attn[HD * h:HD * (h + 1)] * scale)
                bqk.append(b_attn[D + HD * h:D + HD * (h + 1)])
                bv.append(b_attn[2 * D + HD * h:2 * D + HD * (h + 1)])
            m["b_qk"] = np.ascontiguousarray(
                np.concatenate(bqk)[None, :].astype(BF16))
            m["b_v"] = np.ascontiguousarray(
                np.concatenate(bv)[None, :].astype(BF16))
        if with_bproj:
            m["b_proj"] = np.ascontiguousarray(
                np.asarray(b_proj, np.float32)[None, :].astype(BF16))
        in_maps.append(m)
    return in_maps, (with_mask, with_battn, with_bproj)


def _run(inputs, trace=False, tmpdir=None):
    from concourse.bass_utils import run_bass_kernel_spmd

    in_maps, key = _prep_inputs(**inputs)
    if key not in _CACHE:
        _CACHE[key] = _build(*key)
    nc = _CACHE[key]
    try:
        res = run_bass_kernel_spmd(nc, in_maps, core_ids=list(range(NCORES)),
                                   trace=trace, tmpdir=tmpdir)
    except Exception as e:
        if "unrecoverable" not in str(e) and "UNAVAILABLE" not in str(e):
            raise
        import ctypes
        lib = ctypes.CDLL("/opt/axon/libaxon_pjrt.so")
        if hasattr(lib, "axon_reset"):
            lib.axon_reset.restype = ctypes.c_int64
            lib.axon_reset()
        res = run_bass_kernel_spmd(nc, in_maps, core_ids=list(range(NCORES)),
                                   trace=trace, tmpdir=tmpdir)
    y = np.empty((B, T, D), np.float32)
    for c in range(NCORES):
        o = res.results[c]["out"]
        y[0, RHALF * c:RHALF * (c + 1)] = o[:RHALF]
        y[1, RHALF * c:RHALF * (c + 1)] = o[RHALF:]
    return y, res


def kernel(**inputs) -> np.ndarray:
    y, _ = _run(inputs, trace=False)
    return y


# revision 33
# speedup vs baseline: 1.3531x; 1.0795x over previous
"""Distributed multi-head attention kernel for one TRN2 chip (8 NeuronCores).

Problem: B=2, T=2048, D=1024, H=16 heads (hd=64).
  qkv = x @ w_attn + b_attn ; per-head softmax((q k^T)/sqrt(hd) + 2*mask) v
  out = attn @ w_proj + b_proj

Sharding: tensor-parallel over heads. Core c owns heads {2c, 2c+1}.
  - Q/K projections in transposed layout (QT/KT: [hd, T]); head h lives at
    partitions [64h, 64h+64) of qt2/kt2.
  - S^T[kv, q] = K_h^T Q_h as K=64 row-tiled matmul pairs: the two heads
    occupy PE row-halves (tile_position (0,0)/(64,0)) and stream
    concurrently - 2x over the zero-padded K=128 formulation.
  - V is computed directly in natural [t, hd] layout (lhsT = x chunk,
    rhs = w_v), eliminating the PE transposes of the old design.
  - Softmax denominator comes from a ones-column appended to V in the PV
    matmul (O_ext = [V|1]^T @ P^T); PV is stream-bound and runs at the
    N-cycle roofline already.
  - Exp runs on ScalarE only (it is the pacing engine); all PSUM->SBUF
    copies are on VectorE.
  - Mid-kernel AllToAll (two halves, one per batch) reshards from
    head-space to row-space; the final projection then needs no reduction.
  - All matmul operands bf16; accumulation + softmax statistics fp32.
"""

import sys

sys.path.insert(0, "/opt/trn_rl_repo")

import numpy as np

B, T, D = 2, 2048, 1024
H = 16
HD = D // H
NCORES = 8
HPC = H // NCORES          # heads per core = 2
BT = B * T                 # 4096 global rows
ROWS_PER_CORE = BT // NCORES   # 512
RHALF = ROWS_PER_CORE // 2     # 256 rows per A2A half
TB = 512                   # t-block width for QKV projection
NTB = BT // TB             # 8
NKD = D // 128             # 8 contraction chunks over D
QB = 512                   # q-block width in attention
NQB = T // QB              # 4 per (batch, head)
NKV = T // 128             # 16 kv chunks per batch
PVLAG = 6                  # PV trails exp by this many kv chunks

_CACHE = {}
import ml_dtypes
BF16 = ml_dtypes.bfloat16


def _build(with_mask: bool, with_battn: bool, with_bproj: bool):
    import concourse.bass as bass
    import concourse.tile as tile
    from concourse import bacc, mybir

    f32 = mybir.dt.float32
    bf16 = mybir.dt.bfloat16
    Exp = mybir.ActivationFunctionType.Exp

    nc = bacc.Bacc("TRN2", target_bir_lowering=False, debug=False,
                   num_devices=NCORES)
    rg = [list(range(NCORES))]

    xt = nc.dram_tensor("xt", [D, BT], bf16, kind="ExternalInput")
    w_qk = nc.dram_tensor("w_qk", [D, 256], bf16, kind="ExternalInput")
    w_v = nc.dram_tensor("w_v", [D, 128], bf16, kind="ExternalInput")
    w_proj = nc.dram_tensor("w_proj", [D, D], bf16, kind="ExternalInput")
    if with_mask:
        mask2 = nc.dram_tensor("mask2", [128, B * NKV], f32,
                               kind="ExternalInput")
    if with_battn:
        b_qk = nc.dram_tensor("b_qk", [1, 256], bf16, kind="ExternalInput")
        b_v = nc.dram_tensor("b_v", [1, 128], bf16, kind="ExternalInput")
    if with_bproj:
        b_proj = nc.dram_tensor("b_proj", [1, D], bf16, kind="ExternalInput")
    out = nc.dram_tensor("out", [ROWS_PER_CORE, D], f32, kind="ExternalOutput")

    with tile.TileContext(nc, num_cores=NCORES) as tc:
        from contextlib import ExitStack
        with ExitStack() as ctx:
            const = ctx.enter_context(tc.tile_pool(name="const", bufs=1))
            xt_pool = ctx.enter_context(tc.tile_pool(name="xtp", bufs=4))
            qk_pool = ctx.enter_context(tc.tile_pool(name="qkp", bufs=1))
            pt_pool = ctx.enter_context(tc.tile_pool(name="ptp", bufs=9))
            lbc_pool = ctx.enter_context(tc.tile_pool(name="lbc", bufs=2))
            pin_pool = ctx.enter_context(tc.tile_pool(name="pin", bufs=8))
            out_pool = ctx.enter_context(tc.tile_pool(name="outp", bufs=2))
            # [128, 1024] f32 = 2 PSUM banks per slot; 3 slots = 6 banks.
            # One slot holds BOTH heads' S^T chunk (h0 -> bank a, h1 ->
            # bank b), so the row-tiled matmul pair shares one slot
            # dependency and issues back-to-back (concurrent row groups),
            # and exp runs as a single N=1024 ACTIVATE.
            hot = ctx.enter_context(tc.tile_pool(name="hot", bufs=3,
                                                 space="PSUM"))
            acc_pool = ctx.enter_context(tc.tile_pool(name="accp", bufs=2,
                                                      space="PSUM"))
            dram = ctx.enter_context(tc.tile_pool(name="dram", bufs=2,
                                                  space="DRAM"))

            # ---- ACT exp-table preload (runs at t~0 on the scalar queue) --
            dmy = const.tile([1, 8], bf16)
            nc.vector.memset(dmy[:], 0.0)
            dmy2 = const.tile([1, 8], f32)
            nc.scalar.activation(out=dmy2[:], in_=dmy[:], func=Exp)

            # ---- PE warmup: dependency-free matmuls run during the input
            # DMA wait, flipping the HAM clock gate to full rate before the
            # first real QKV matmuls issue (saves ~1/2-rate on the first
            # ~16 matmuls of the pipeline)
            wrm = const.tile([128, TB], bf16)
            nc.vector.memset(wrm[:], 0.0)
            wps = hot.tile([128, 2 * TB], f32, tag="hot", name="wps")
            for i in range(16):
                nc.tensor.matmul(
                    wps[:, TB * (i % 2):TB * (i % 2 + 1)],
                    lhsT=wrm[:, 0:128], rhs=wrm[:],
                    start=True, stop=True)

            # ---- constants ----
            # w_qk packed per D-chunk: [128, NKD, 256]; group g cols
            # [128g, 128g+128): g0=[q_h0/8|k_h0] g1=[q_h1/8|k_h1]
            wqk_sb = const.tile([128, NKD, 256], bf16)
            for half in range(2):
                nc.sync.dma_start(
                    out=wqk_sb[:, 4 * half:4 * (half + 1), :],
                    in_=w_qk[:].rearrange("(a p) c -> p a c", p=128)[
                        :, 4 * half:4 * (half + 1), :])
            # w_v packed per D-chunk: [128, NKD, 128]; cols [v_h0|v_h1]
            wv_sb = const.tile([128, NKD, 128], bf16)
            nc.scalar.dma_start(
                out=wv_sb[:],
                in_=w_v[:].rearrange("(a p) c -> p a c", p=128))
            wproj_sb = const.tile([128, NKD, D], bf16)

            def emit_wproj_loads():
                for half in range(2):
                    nc.sync.dma_start(
                        out=wproj_sb[:, 4 * half:4 * (half + 1), :],
                        in_=w_proj[:].rearrange("(a p) c -> p a c", p=128)[
                            :, 4 * half:4 * (half + 1), :])

            if with_mask:
                # mask (already doubled on host): [128, B, NKV]
                mask_sb = const.tile([128, B, NKV], f32)
                nc.sync.dma_start(out=mask_sb[:],
                                  in_=mask2[:].rearrange("p (b j) -> p b j",
                                                         b=B))
            if with_battn:
                bqk_sb = const.tile([1, 256], bf16)
                nc.sync.dma_start(out=bqk_sb[:], in_=b_qk[:])
                bv_sb = const.tile([1, 128], bf16)
                nc.sync.dma_start(out=bv_sb[:], in_=b_v[:])
                ones_row = const.tile([1, TB], bf16)
                nc.vector.memset(ones_row[:], 1.0)
            if with_bproj:
                bproj_sb = const.tile([1, D], bf16)
                nc.sync.dma_start(out=bproj_sb[:], in_=b_proj[:])
                ones_col = const.tile([1, 128], bf16)
                nc.vector.memset(ones_col[:], 1.0)

            ones64 = const.tile([1, 64], bf16)
            nc.vector.memset(ones64[:], 1.0)

            # persistent activations. Head h at partitions [64h, 64h+64).
            qt2 = qk_pool.tile([128, BT], bf16, tag="qt2", name="qt2")
            kt2 = qk_pool.tile([128, BT], bf16, tag="kt2", name="kt2")
            ot = qk_pool.tile([128, BT], bf16, tag="ot", name="ot")
            # V natural layout + ones column: subtile s = 32b + 2j + h is
            # [128 kv, 65] = [V_chunk | 1].
            vnat = qk_pool.tile([128, 2 * NKV * HPC, 65], bf16, tag="vnat",
                                name="vnat")
            nc.vector.memset(vnat[:, :, 64:65], 1.0)

            # ---- QKV projection ----
            # q/k: transposed outputs via stationary weights.
            # v: natural output via stationary x chunks.
            def qkv_tblock_units(tb):
                xt_t = xt_pool.tile([128, NKD, TB], bf16, tag="xt", name="xt")

                def dma_unit():
                    xsrc = xt[:].rearrange("(a p) t -> p a t", p=128)
                    for half in range(2):
                        nc.sync.dma_start(
                            out=xt_t[:, 4 * half:4 * (half + 1), :],
                            in_=xsrc[:, 4 * half:4 * (half + 1),
                                     TB * tb:TB * (tb + 1)])

                def qk_unit(g):
                    ps = hot.tile([128, 2 * TB], f32, tag="hot", name="qkps")
                    for d in range(NKD):
                        nc.tensor.matmul(
                            ps[:, 0:TB],
                            lhsT=wqk_sb[:, d, 128 * g:128 * (g + 1)],
                            rhs=xt_t[:, d, :],
                            start=(d == 0),
                            stop=(d == NKD - 1) and not with_battn)
                    if with_battn:
                        nc.tensor.matmul(
                            ps[:, 0:TB],
                            lhsT=bqk_sb[:, 128 * g:128 * (g + 1)],
                            rhs=ones_row[:],
                            start=False, stop=True)
                    cs = TB * tb
                    nc.vector.tensor_copy(
                        out=qt2[64 * g:64 * (g + 1), cs:cs + TB],
                        in_=ps[0:64, 0:TB])
                    nc.vector.tensor_copy(
                        out=kt2[64 * g:64 * (g + 1), cs:cs + TB],
                        in_=ps[64:128, 0:TB])

                def v_unit(vh):
                    # two 128-row t-chunks per unit
                    ps = hot.tile([128, 2 * TB], f32, tag="hot", name="vps")
                    for ch in range(2):
                        c4 = 2 * vh + ch
                        for d in range(NKD):
                            nc.tensor.matmul(
                                ps[:, 128 * ch:128 * (ch + 1)],
                                lhsT=xt_t[:, d, 128 * c4:128 * (c4 + 1)],
                                rhs=wv_sb[:, d, :],
                                start=(d == 0),
                                stop=(d == NKD - 1) and not with_battn)
                        if with_battn:
                            nc.tensor.matmul(
                                ps[:, 128 * ch:128 * (ch + 1)],
                                lhsT=ones_row[:, 0:128],
                                rhs=bv_sb[:],
                                start=False, stop=True)
                    for ch in range(2):
                        c4 = 2 * vh + ch
                        gj = 4 * tb + c4          # global 128-row chunk id
                        b_, j = gj // NKV, gj % NKV
                        for h in range(HPC):
                            s = 32 * b_ + 2 * j + h
                            nc.vector.tensor_copy(
                                out=vnat[:, s, 0:64],
                                in_=ps[:, 128 * ch + 64 * h:
                                       128 * ch + 64 * (h + 1)])

                # finer-grained halves so PE-queue fillers never stall
                # the scalar engine for more than ~1us
                qkps = {}

                def qk_half(g, half):
                    if g not in qkps:
                        qkps[g] = hot.tile([128, 2 * TB], f32, tag="hot",
                                           name="qkps")
                    ps = qkps[g]
                    for d in range(4 * half, 4 * half + 4):
                        nc.tensor.matmul(
                            ps[:, 0:TB],
                            lhsT=wqk_sb[:, d, 128 * g:128 * (g + 1)],
                            rhs=xt_t[:, d, :],
                            start=(d == 0),
                            stop=(d == NKD - 1) and not with_battn)
                    if half == 0:
                        return
                    if with_battn:
                        nc.tensor.matmul(
                            ps[:, 0:TB],
                            lhsT=bqk_sb[:, 128 * g:128 * (g + 1)],
                            rhs=ones_row[:],
                            start=False, stop=True)
                    cs = TB * tb
                    nc.vector.tensor_copy(
                        out=qt2[64 * g:64 * (g + 1), cs:cs + TB],
                        in_=ps[0:64, 0:TB])
                    nc.vector.tensor_copy(
                        out=kt2[64 * g:64 * (g + 1), cs:cs + TB],
                        in_=ps[64:128, 0:TB])
                    del qkps[g]

                def v_chunk(c4):
                    ps = hot.tile([128, 2 * TB], f32, tag="hot", name="vps")
                    for d in range(NKD):
                        nc.tensor.matmul(
                            ps[:, 0:128],
                            lhsT=xt_t[:, d, 128 * c4:128 * (c4 + 1)],
                            rhs=wv_sb[:, d, :],
                            start=(d == 0),
                            stop=(d == NKD - 1) and not with_battn)
                    if with_battn:
                        nc.tensor.matmul(
                            ps[:, 0:128],
                            lhsT=ones_row[:, 0:128],
                            rhs=bv_sb[:],
                            start=False, stop=True)
                    gj = 4 * tb + c4          # global 128-row chunk id
                    b_, j = gj // NKV, gj % NKV
                    for h in range(HPC):
                        s = 32 * b_ + 2 * j + h
                        nc.vector.tensor_copy(
                            out=vnat[:, s, 0:64],
                            in_=ps[:, 64 * h:64 * (h + 1)])

                return [dma_unit,
                        lambda: qk_half(0, 0), lambda: qk_half(0, 1),
                        lambda: qk_half(1, 0), lambda: qk_half(1, 1),
                        lambda: v_chunk(0), lambda: v_chunk(1),
                        lambda: v_chunk(2), lambda: v_chunk(3)]

            # ---- attention: one global software-pipelined chunk stream ----
            # chunk g = (b, qb, j); S^T+exp at position g, PV at g+PVLAG.
            # The PV lag spans q-block boundaries so the normalize of
            # q-block Q overlaps the S^T/exp flow of Q+1.
            CHUNKS = [(b, qb, j) for b in range(B) for qb in range(NQB)
                      for j in range(NKV)]
            pts = {}
            ps_o_by_q = {}

            def emit_st(g):
                b, qb, j = CHUNKS[g]
                c0 = 2048 * b + QB * qb
                k0 = 2048 * b + 128 * j
                # both heads' S^T chunk in one 2-bank slot; the two K=64
                # row-tiled matmuls hit disjoint PE row groups and
                # disjoint banks -> concurrent
                st = hot.tile([128, 2 * QB], f32, tag="hot", name="st")
                for h in range(HPC):
                    nc.tensor.matmul(
                        st[:, QB * h:QB * (h + 1)],
                        lhsT=kt2[64 * h:64 * (h + 1), k0:k0 + 128],
                        rhs=qt2[64 * h:64 * (h + 1), c0:c0 + QB],
                        start=True, stop=True)
                pt = pt_pool.tile([128, 2 * QB], bf16, tag="pt", name="pt")
                if with_mask:
                    nc.scalar.activation(out=pt[:], in_=st[:], func=Exp,
                                         bias=mask_sb[:, b, j:j + 1],
                                         scale=1.0)
                else:
                    nc.scalar.activation(out=pt[:], in_=st[:], func=Exp)
                pts[g] = pt

            def emit_pv(g):
                b, qb, j = CHUNKS[g]
                if j == 0:
                    ps_o_by_q[(b, qb)] = [
                        acc_pool.tile([65, QB], f32, tag="acc", name="acc")
                        for _ in range(HPC)]
                ps_o = ps_o_by_q[(b, qb)]
                pt = pts.pop(g)
                for h in range(HPC):
                    s = 32 * b + 2 * j + h
                    nc.tensor.matmul(
                        ps_o[h][:],
                        lhsT=vnat[:, s, :],
                        rhs=pt[:, QB * h:QB * (h + 1)],
                        start=(j == 0), stop=(j == NKV - 1),
                        skip_group_check=True)
                if j == NKV - 1:
                    normalize(b, qb)

            def normalize(b, qb):
                # drain ps_o to SBUF with two proven-shape copies (64 and
                # 1 partitions) so the 2 accumulator banks recycle fast;
                # recip + DRAM broadcast roundtrip + mul run off the
                # critical path on SBUF data
                c0 = 2048 * b + QB * qb
                ps_o = ps_o_by_q.pop((b, qb))
                for h in range(HPC):
                    osum = lbc_pool.tile([64, QB], f32, tag="osum",
                                         name="osum")
                    nc.vector.tensor_copy(out=osum[:], in_=ps_o[h][0:64, :])
                    lsb = lbc_pool.tile([1, QB], f32, tag="lsb", name="lsb")
                    nc.vector.tensor_copy(out=lsb[:], in_=ps_o[h][64:65, :])
                    lrec = lbc_pool.tile([1, QB], f32, tag="lrec",
                                         name="lrec")
                    nc.vector.reciprocal_approx_fast(
                        out=lrec[:], in_=lsb[:])
                    ldram = dram.tile([1, QB], f32, tag="ld", name="ld")
                    nc.sync.dma_start(out=ldram[:], in_=lrec[:])
                    lbc = lbc_pool.tile([64, QB], f32, tag="lbc", name="lbc")
                    nc.sync.dma_start(out=lbc[:],
                                      in_=ldram[:].to_broadcast([64, QB]))
                    nc.vector.tensor_mul(
                        out=ot[64 * h:64 * (h + 1), c0:c0 + QB],
                        in0=osum[:], in1=lbc[:])
                a2a_stage(b, qb)

            # ---- A2A half (reshard head-space -> row-space) ----
            a2a_outs = {}
            a2a_ins = {}

            def a2a_stage(half, qb):
                if half not in a2a_ins:
                    a2a_ins[half] = dram.tile([NCORES, 128, RHALF], bf16,
                                              tag="a2ain", name="a2ain")
                a_in = a2a_ins[half]
                for j in (2 * qb, 2 * qb + 1):
                    nc.sync.dma_start(
                        out=a_in[j],
                        in_=ot[:, 2048 * half + RHALF * j:
                               2048 * half + RHALF * (j + 1)])

            def a2a_half(half):
                from concourse import mybir as _mb
                a_in = a2a_ins[half]
                a_out = dram.tile([NCORES, 128, RHALF], bf16, tag="a2aout",
                                  name="a2aout")
                nc.gpsimd.collective_compute(
                    "AllToAll", _mb.AluOpType.bypass, replica_groups=rg,
                    ins=[a_in.opt()], outs=[a_out.opt()])
                a2a_outs[half] = a_out

            pins_by_half = {}

            def pins_load(half):
                # on the gpsimd queue so the wait-for-collective does not
                # head-of-line-block the sync queue's normalize DMAs
                a_out = a2a_outs[half]
                pins = []
                for j in range(NCORES):
                    p = pin_pool.tile([128, RHALF], bf16, tag="pin",
                                      name="pin")
                    nc.gpsimd.dma_start(out=p[:], in_=a_out[j])
                    pins.append(p)
                pins_by_half[half] = pins

            proj_state = {}

            def proj_unit(half, rb, n):
                pins = pins_by_half[half]
                if (half, rb) not in proj_state:
                    proj_state[(half, rb)] = (
                        out_pool.tile([128, D], f32, tag="osb", name="osb"),
                        hot.tile([128, 1024], f32, tag="hot", name="pjps"))
                osb, ps = proj_state[(half, rb)]
                for j in range(NCORES):
                    nc.tensor.matmul(
                        ps[:, 512 * n:512 * (n + 1)],
                        lhsT=pins[j][:, 128 * rb:128 * (rb + 1)],
                        rhs=wproj_sb[:, j, 512 * n:512 * (n + 1)],
                        start=(j == 0),
                        stop=(j == NCORES - 1) and not with_bproj)
                if with_bproj:
                    nc.tensor.matmul(
                        ps[:, 512 * n:512 * (n + 1)], lhsT=ones_col[:],
                        rhs=bproj_sb[:, 512 * n:512 * (n + 1)],
                        start=False, stop=True)
                nc.vector.tensor_copy(out=osb[:, 512 * n:512 * (n + 1)],
                                      in_=ps[:, 512 * n:512 * (n + 1)])
                if n == 1:
                    del proj_state[(half, rb)]
                    r0 = 256 * half + 128 * rb
                    nc.sync.dma_start(out=out[r0:r0 + 128, :], in_=osb[:])

            def proj_rb(half, rb):
                proj_unit(half, rb, 0)
                proj_unit(half, rb, 1)

            # ---- emission order ----
            # u[tb] = [dma, qk0a, qk0b, qk1a, qk1b, v0, v1, v2, v3]
            u = {tb: qkv_tblock_units(tb) for tb in range(NTB)}
            for f in u[0][:5]:    # tb0 dma+qk only; v deferred so the
                f()               # first S^T isn't queued behind it
            fill = {}

            def put(g, *fns):
                fill.setdefault(g, []).extend(fns)

            # b0 phase: just-in-time qk for tb1-3 (kt chunk 4j needed by
            # chunk 4j), V units placed PVLAG chunks before their PV
            put(0, u[0][5])
            put(1, u[1][0], u[1][1])
            put(2, u[1][2], u[1][3])
            put(3, u[1][4], u[0][6])
            put(4, u[0][7], u[0][8])
            put(5, u[2][0], u[1][5])
            put(6, u[2][1], u[2][2])
            put(7, u[2][3], u[2][4])
            put(8, u[1][6], u[1][7])
            put(9, u[3][0], u[1][8])
            put(10, u[3][1], u[3][2])
            put(11, u[3][3], u[3][4])
            put(12, u[2][5], u[2][6])
            put(13, u[2][7], u[2][8])
            put(14, u[3][5], u[3][6])
            put(15, u[3][7], u[3][8])
            # b1 QKV: no urgency, one unit per chunk
            for i, tb in enumerate((4, 5, 6, 7)):
                g0 = 18 + 10 * i
                for k in range(9):
                    put(g0 + k, u[tb][k])
            put(57, emit_wproj_loads)
            put(76, lambda: a2a_half(0))
            put(82, lambda: pins_load(0))

            NG = len(CHUNKS)
            for g in range(NG + PVLAG):
                if g in fill:
                    for fn in fill[g]:
                        fn()
                if g < NG:
                    emit_st(g)
                if g >= PVLAG:
                    emit_pv(g - PVLAG)
            # final reshard + projection; proj(0) runs inside the second
            # collective's window, with dummy matmuls sprinkled in to keep
            # the PE's HAM clock warm until proj(1)'s inputs arrive
            def dummy_mms(n):
                ps = hot.tile([128, 1024], f32, tag="hot", name="warm")
                for i in range(n):
                    nc.tensor.matmul(
                        ps[:, 512 * (i % 2):512 * (i % 2 + 1)],
                        lhsT=wproj_sb[:, 0, 0:128],
                        rhs=wproj_sb[:, 1, 0:512],
                        start=True, stop=True)

            a2a_half(1)
            dummy_mms(4)
            proj_unit(0, 0, 0)
            dummy_mms(2)
            proj_unit(0, 0, 1)
            dummy_mms(2)
            proj_unit(0, 1, 0)
            dummy_mms(2)
            proj_unit(0, 1, 1)
            dummy_mms(24)
            pins_load(1)
            proj_rb(1, 0)
            proj_rb(1, 1)

    nc.finalize()
    return nc


def _prep_inputs(x, attention_mask, w_attn, b_attn, w_proj, b_proj):
    x = np.asarray(x, np.float32)
    xt = np.ascontiguousarray(x.reshape(BT, D).T).astype(BF16)
    w_attn = np.asarray(w_attn, np.float32)
    b_attn = np.asarray(b_attn, np.float32)
    wp = np.ascontiguousarray(np.asarray(w_proj, np.float32)).astype(BF16)
    scale = 1.0 / np.sqrt(HD)
    am = np.asarray(attention_mask, np.float32)
    with_mask = bool(np.any(am))
    with_battn = bool(np.any(b_attn))
    with_bproj = bool(np.any(np.asarray(b_proj)))
    mask2 = None
    if with_mask:
        m2 = (2.0 * am).reshape(B, T // 128, 128)
        mask2 = np.ascontiguousarray(m2.transpose(2, 0, 1).reshape(128, -1))
    in_maps = []
    for c in range(NCORES):
        h0, h1 = HPC * c, HPC * c + 1
        qkcols = []
        vcols = []
        for h in (h0, h1):
            qkcols.append(w_attn[:, HD * h:HD * (h + 1)] * scale)      # q
            qkcols.append(w_attn[:, D + HD * h:D + HD * (h + 1)])      # k
            vcols.append(w_attn[:, 2 * D + HD * h:2 * D + HD * (h + 1)])
        wqk = np.ascontiguousarray(np.concatenate(qkcols, axis=1)).astype(BF16)
        wv = np.ascontiguousarray(np.concatenate(vcols, axis=1)).astype(BF16)
        m = {"xt": xt, "w_qk": wqk, "w_v": wv, "w_proj": wp}
        if with_mask:
            m["mask2"] = mask2
        if with_battn:
            bqk = []
            bv = []
            for h in (h0, h1):
                bqk.append(b_attn[HD * h:HD * (h + 1)] * scale)
                bqk.append(b_attn[D + HD * h:D + HD * (h + 1)])
                bv.append(b_attn[2 * D + HD * h:2 * D + HD * (h + 1)])
            m["b_qk"] = np.ascontiguousarray(
                np.concatenate(bqk)[None, :].astype(BF16))
            m["b_v"] = np.ascontiguousarray(
                np.concatenate(bv)[None, :].astype(BF16))
        if with_bproj:
            m["b_proj"] = np.ascontiguousarray(
                np.asarray(b_proj, np.float32)[None, :].astype(BF16))
        in_maps.append(m)
    return in_maps, (with_mask, with_battn, with_bproj)


def _run(inputs, trace=False, tmpdir=None):
    from concourse.bass_utils import run_bass_kernel_spmd

    in_maps, key = _prep_inputs(**inputs)
    if key not in _CACHE:
        _CACHE[key] = _build(*key)
    nc = _CACHE[key]
    try:
        res = run_bass_kernel_spmd(nc, in_maps, core_ids=list(range(NCORES)),
                                   trace=trace, tmpdir=tmpdir)
    except Exception as e:
        if "unrecoverable" not in str(e) and "UNAVAILABLE" not in str(e):
            raise
        import ctypes
        lib = ctypes.CDLL("/opt/axon/libaxon_pjrt.so")
        if hasattr(lib, "axon_reset"):
            lib.axon_reset.restype = ctypes.c_int64
            lib.axon_reset()
        res = run_bass_kernel_spmd(nc, in_maps, core_ids=list(range(NCORES)),
                                   trace=trace, tmpdir=tmpdir)
    y = np.empty((B, T, D), np.float32)
    for c in range(NCORES):
        o = res.results[c]["out"]
        y[0, RHALF * c:RHALF * (c + 1)] = o[:RHALF]
        y[1, RHALF * c:RHALF * (c + 1)] = o[RHALF:]
    return y, res


def kernel(**inputs) -> np.ndarray:
    y, _ = _run(inputs, trace=False)
    return y


# revision 34
# speedup vs baseline: 1.3572x; 1.0030x over previous
"""Distributed multi-head attention kernel for one TRN2 chip (8 NeuronCores).

Problem: B=2, T=2048, D=1024, H=16 heads (hd=64).
  qkv = x @ w_attn + b_attn ; per-head softmax((q k^T)/sqrt(hd) + 2*mask) v
  out = attn @ w_proj + b_proj

Sharding: tensor-parallel over heads. Core c owns heads {2c, 2c+1}.
  - Q/K projections in transposed layout (QT/KT: [hd, T]); head h lives at
    partitions [64h, 64h+64) of qt2/kt2.
  - S^T[kv, q] = K_h^T Q_h as K=64 row-tiled matmul pairs: the two heads
    occupy PE row-halves (tile_position (0,0)/(64,0)) and stream
    concurrently - 2x over the zero-padded K=128 formulation.
  - V is computed directly in natural [t, hd] layout (lhsT = x chunk,
    rhs = w_v), eliminating the PE transposes of the old design.
  - Softmax denominator comes from a ones-column appended to V in the PV
    matmul (O_ext = [V|1]^T @ P^T); PV is stream-bound and runs at the
    N-cycle roofline already.
  - Exp runs on ScalarE only (it is the pacing engine); all PSUM->SBUF
    copies are on VectorE.
  - Mid-kernel AllToAll (two halves, one per batch) reshards from
    head-space to row-space; the final projection then needs no reduction.
  - All matmul operands bf16; accumulation + softmax statistics fp32.
"""

import sys

sys.path.insert(0, "/opt/trn_rl_repo")

import numpy as np

B, T, D = 2, 2048, 1024
H = 16
HD = D // H
NCORES = 8
HPC = H // NCORES          # heads per core = 2
BT = B * T                 # 4096 global rows
ROWS_PER_CORE = BT // NCORES   # 512
RHALF = ROWS_PER_CORE // 2     # 256 rows per A2A half
TB = 512                   # t-block width for QKV projection
NTB = BT // TB             # 8
NKD = D // 128             # 8 contraction chunks over D
QB = 512                   # q-block width in attention
NQB = T // QB              # 4 per (batch, head)
NKV = T // 128             # 16 kv chunks per batch
PVLAG = 6                  # PV trails exp by this many kv chunks

_CACHE = {}
import ml_dtypes
BF16 = ml_dtypes.bfloat16


def _build(with_mask: bool, with_battn: bool, with_bproj: bool):
    import concourse.bass as bass
    import concourse.tile as tile
    from concourse import bacc, mybir

    f32 = mybir.dt.float32
    bf16 = mybir.dt.bfloat16
    Exp = mybir.ActivationFunctionType.Exp

    nc = bacc.Bacc("TRN2", target_bir_lowering=False, debug=False,
                   num_devices=NCORES)
    rg = [list(range(NCORES))]

    xt = nc.dram_tensor("xt", [D, BT], bf16, kind="ExternalInput")
    w_qk = nc.dram_tensor("w_qk", [D, 256], bf16, kind="ExternalInput")
    w_v = nc.dram_tensor("w_v", [D, 128], bf16, kind="ExternalInput")
    w_proj = nc.dram_tensor("w_proj", [D, D], bf16, kind="ExternalInput")
    if with_mask:
        mask2 = nc.dram_tensor("mask2", [128, B * NKV], f32,
                               kind="ExternalInput")
    if with_battn:
        b_qk = nc.dram_tensor("b_qk", [1, 256], bf16, kind="ExternalInput")
        b_v = nc.dram_tensor("b_v", [1, 128], bf16, kind="ExternalInput")
    if with_bproj:
        b_proj = nc.dram_tensor("b_proj", [1, D], bf16, kind="ExternalInput")
    out = nc.dram_tensor("out", [ROWS_PER_CORE, D], f32, kind="ExternalOutput")

    with tile.TileContext(nc, num_cores=NCORES) as tc:
        from contextlib import ExitStack
        with ExitStack() as ctx:
            const = ctx.enter_context(tc.tile_pool(name="const", bufs=1))
            xt_pool = ctx.enter_context(tc.tile_pool(name="xtp", bufs=4))
            qk_pool = ctx.enter_context(tc.tile_pool(name="qkp", bufs=1))
            pt_pool = ctx.enter_context(tc.tile_pool(name="ptp", bufs=9))
            lbc_pool = ctx.enter_context(tc.tile_pool(name="lbc", bufs=2))
            pin_pool = ctx.enter_context(tc.tile_pool(name="pin", bufs=8))
            out_pool = ctx.enter_context(tc.tile_pool(name="outp", bufs=2))
            # [128, 1024] f32 = 2 PSUM banks per slot; 3 slots = 6 banks.
            # One slot holds BOTH heads' S^T chunk (h0 -> bank a, h1 ->
            # bank b), so the row-tiled matmul pair shares one slot
            # dependency and issues back-to-back (concurrent row groups),
            # and exp runs as a single N=1024 ACTIVATE.
            hot = ctx.enter_context(tc.tile_pool(name="hot", bufs=3,
                                                 space="PSUM"))
            acc_pool = ctx.enter_context(tc.tile_pool(name="accp", bufs=2,
                                                      space="PSUM"))
            dram = ctx.enter_context(tc.tile_pool(name="dram", bufs=2,
                                                  space="DRAM"))

            # ---- ACT exp-table preload (runs at t~0 on the scalar queue) --
            dmy = const.tile([1, 8], bf16)
            nc.vector.memset(dmy[:], 0.0)
            dmy2 = const.tile([1, 8], f32)
            nc.scalar.activation(out=dmy2[:], in_=dmy[:], func=Exp)

            # ---- PE warmup: dependency-free matmuls run during the input
            # DMA wait, flipping the HAM clock gate to full rate before the
            # first real QKV matmuls issue (saves ~1/2-rate on the first
            # ~16 matmuls of the pipeline)
            wrm = const.tile([128, TB], bf16)
            nc.vector.memset(wrm[:], 0.0)
            wps = hot.tile([128, 2 * TB], f32, tag="hot", name="wps")
            for i in range(28):
                nc.tensor.matmul(
                    wps[:, TB * (i % 2):TB * (i % 2 + 1)],
                    lhsT=wrm[:, 0:128], rhs=wrm[:],
                    start=True, stop=True)

            # ---- constants ----
            # w_qk packed per D-chunk: [128, NKD, 256]; group g cols
            # [128g, 128g+128): g0=[q_h0/8|k_h0] g1=[q_h1/8|k_h1]
            wqk_sb = const.tile([128, NKD, 256], bf16)
            for half in range(2):
                nc.sync.dma_start(
                    out=wqk_sb[:, 4 * half:4 * (half + 1), :],
                    in_=w_qk[:].rearrange("(a p) c -> p a c", p=128)[
                        :, 4 * half:4 * (half + 1), :])
            # w_v packed per D-chunk: [128, NKD, 128]; cols [v_h0|v_h1]
            wv_sb = const.tile([128, NKD, 128], bf16)
            nc.scalar.dma_start(
                out=wv_sb[:],
                in_=w_v[:].rearrange("(a p) c -> p a c", p=128))
            wproj_sb = const.tile([128, NKD, D], bf16)

            def emit_wproj_loads():
                for half in range(2):
                    nc.sync.dma_start(
                        out=wproj_sb[:, 4 * half:4 * (half + 1), :],
                        in_=w_proj[:].rearrange("(a p) c -> p a c", p=128)[
                            :, 4 * half:4 * (half + 1), :])

            if with_mask:
                # mask (already doubled on host): [128, B, NKV]
                mask_sb = const.tile([128, B, NKV], f32)
                nc.sync.dma_start(out=mask_sb[:],
                                  in_=mask2[:].rearrange("p (b j) -> p b j",
                                                         b=B))
            if with_battn:
                bqk_sb = const.tile([1, 256], bf16)
                nc.sync.dma_start(out=bqk_sb[:], in_=b_qk[:])
                bv_sb = const.tile([1, 128], bf16)
                nc.sync.dma_start(out=bv_sb[:], in_=b_v[:])
                ones_row = const.tile([1, TB], bf16)
                nc.vector.memset(ones_row[:], 1.0)
            if with_bproj:
                bproj_sb = const.tile([1, D], bf16)
                nc.sync.dma_start(out=bproj_sb[:], in_=b_proj[:])
                ones_col = const.tile([1, 128], bf16)
                nc.vector.memset(ones_col[:], 1.0)

            ones64 = const.tile([1, 64], bf16)
            nc.vector.memset(ones64[:], 1.0)

            # persistent activations. Head h at partitions [64h, 64h+64).
            qt2 = qk_pool.tile([128, BT], bf16, tag="qt2", name="qt2")
            kt2 = qk_pool.tile([128, BT], bf16, tag="kt2", name="kt2")
            ot = qk_pool.tile([128, BT], bf16, tag="ot", name="ot")
            # V natural layout + ones column: subtile s = 32b + 2j + h is
            # [128 kv, 65] = [V_chunk | 1].
            vnat = qk_pool.tile([128, 2 * NKV * HPC, 65], bf16, tag="vnat",
                                name="vnat")
            nc.vector.memset(vnat[:, :, 64:65], 1.0)

            # ---- QKV projection ----
            # q/k: transposed outputs via stationary weights.
            # v: natural output via stationary x chunks.
            def qkv_tblock_units(tb):
                xt_t = xt_pool.tile([128, NKD, TB], bf16, tag="xt", name="xt")

                def dma_unit():
                    xsrc = xt[:].rearrange("(a p) t -> p a t", p=128)
                    for half in range(2):
                        nc.sync.dma_start(
                            out=xt_t[:, 4 * half:4 * (half + 1), :],
                            in_=xsrc[:, 4 * half:4 * (half + 1),
                                     TB * tb:TB * (tb + 1)])

                def qk_unit(g):
                    ps = hot.tile([128, 2 * TB], f32, tag="hot", name="qkps")
                    for d in range(NKD):
                        nc.tensor.matmul(
                            ps[:, 0:TB],
                            lhsT=wqk_sb[:, d, 128 * g:128 * (g + 1)],
                            rhs=xt_t[:, d, :],
                            start=(d == 0),
                            stop=(d == NKD - 1) and not with_battn)
                    if with_battn:
                        nc.tensor.matmul(
                            ps[:, 0:TB],
                            lhsT=bqk_sb[:, 128 * g:128 * (g + 1)],
                            rhs=ones_row[:],
                            start=False, stop=True)
                    cs = TB * tb
                    nc.vector.tensor_copy(
                        out=qt2[64 * g:64 * (g + 1), cs:cs + TB],
                        in_=ps[0:64, 0:TB])
                    nc.vector.tensor_copy(
                        out=kt2[64 * g:64 * (g + 1), cs:cs + TB],
                        in_=ps[64:128, 0:TB])

                def v_unit(vh):
                    # two 128-row t-chunks per unit
                    ps = hot.tile([128, 2 * TB], f32, tag="hot", name="vps")
                    for ch in range(2):
                        c4 = 2 * vh + ch
                        for d in range(NKD):
                            nc.tensor.matmul(
                                ps[:, 128 * ch:128 * (ch + 1)],
                                lhsT=xt_t[:, d, 128 * c4:128 * (c4 + 1)],
                                rhs=wv_sb[:, d, :],
                                start=(d == 0),
                                stop=(d == NKD - 1) and not with_battn)
                        if with_battn:
                            nc.tensor.matmul(
                                ps[:, 128 * ch:128 * (ch + 1)],
                                lhsT=ones_row[:, 0:128],
                                rhs=bv_sb[:],
                                start=False, stop=True)
                    for ch in range(2):
                        c4 = 2 * vh + ch
                        gj = 4 * tb + c4          # global 128-row chunk id
                        b_, j = gj // NKV, gj % NKV
                        for h in range(HPC):
                            s = 32 * b_ + 2 * j + h
                            nc.vector.tensor_copy(
                                out=vnat[:, s, 0:64],
                                in_=ps[:, 128 * ch + 64 * h:
                                       128 * ch + 64 * (h + 1)])

                # finer-grained halves so PE-queue fillers never stall
                # the scalar engine for more than ~1us
                qkps = {}

                def qk_half(g, half):
                    if g not in qkps:
                        qkps[g] = hot.tile([128, 2 * TB], f32, tag="hot",
                                           name="qkps")
                    ps = qkps[g]
                    for d in range(4 * half, 4 * half + 4):
                        nc.tensor.matmul(
                            ps[:, 0:TB],
                            lhsT=wqk_sb[:, d, 128 * g:128 * (g + 1)],
                            rhs=xt_t[:, d, :],
                            start=(d == 0),
                            stop=(d == NKD - 1) and not with_battn)
                    if half == 0:
                        return
                    if with_battn:
                        nc.tensor.matmul(
                            ps[:, 0:TB],
                            lhsT=bqk_sb[:, 128 * g:128 * (g + 1)],
                            rhs=ones_row[:],
                            start=False, stop=True)
                    cs = TB * tb
                    nc.vector.tensor_copy(
                        out=qt2[64 * g:64 * (g + 1), cs:cs + TB],
                        in_=ps[0:64, 0:TB])
                    nc.vector.tensor_copy(
                        out=kt2[64 * g:64 * (g + 1), cs:cs + TB],
                        in_=ps[64:128, 0:TB])
                    del qkps[g]

                def v_chunk(c4):
                    ps = hot.tile([128, 2 * TB], f32, tag="hot", name="vps")
                    for d in range(NKD):
                        nc.tensor.matmul(
                            ps[:, 0:128],
                            lhsT=xt_t[:, d, 128 * c4:128 * (c4 + 1)],
                            rhs=wv_sb[:, d, :],
                            start=(d == 0),
                            stop=(d == NKD - 1) and not with_battn)
                    if with_battn:
                        nc.tensor.matmul(
                            ps[:, 0:128],
                            lhsT=ones_row[:, 0:128],
                            rhs=bv_sb[:],
                            start=False, stop=True)
                    gj = 4 * tb + c4          # global 128-row chunk id
                    b_, j = gj // NKV, gj % NKV
                    for h in range(HPC):
                        s = 32 * b_ + 2 * j + h
                        nc.vector.tensor_copy(
                            out=vnat[:, s, 0:64],
                            in_=ps[:, 64 * h:64 * (h + 1)])

                return [dma_unit,
                        lambda: qk_half(0, 0), lambda: qk_half(0, 1),
                        lambda: qk_half(1, 0), lambda: qk_half(1, 1),
                        lambda: v_chunk(0), lambda: v_chunk(1),
                        lambda: v_chunk(2), lambda: v_chunk(3)]

            # ---- attention: one global software-pipelined chunk stream ----
            # chunk g = (b, qb, j); S^T+exp at position g, PV at g+PVLAG.
            # The PV lag spans q-block boundaries so the normalize of
            # q-block Q overlaps the S^T/exp flow of Q+1.
            CHUNKS = [(b, qb, j) for b in range(B) for qb in range(NQB)
                      for j in range(NKV)]
            pts = {}
            ps_o_by_q = {}

            def emit_st(g):
                b, qb, j = CHUNKS[g]
                c0 = 2048 * b + QB * qb
                k0 = 2048 * b + 128 * j
                # both heads' S^T chunk in one 2-bank slot; the two K=64
                # row-tiled matmuls hit disjoint PE row groups and
                # disjoint banks -> concurrent
                st = hot.tile([128, 2 * QB], f32, tag="hot", name="st")
                for h in range(HPC):
                    nc.tensor.matmul(
                        st[:, QB * h:QB * (h + 1)],
                        lhsT=kt2[64 * h:64 * (h + 1), k0:k0 + 128],
                        rhs=qt2[64 * h:64 * (h + 1), c0:c0 + QB],
                        start=True, stop=True)
                pt = pt_pool.tile([128, 2 * QB], bf16, tag="pt", name="pt")
                if with_mask:
                    nc.scalar.activation(out=pt[:], in_=st[:], func=Exp,
                                         bias=mask_sb[:, b, j:j + 1],
                                         scale=1.0)
                else:
                    nc.scalar.activation(out=pt[:], in_=st[:], func=Exp)
                pts[g] = pt

            def emit_pv(g):
                b, qb, j = CHUNKS[g]
                if j == 0:
                    ps_o_by_q[(b, qb)] = [
                        acc_pool.tile([65, QB], f32, tag="acc", name="acc")
                        for _ in range(HPC)]
                ps_o = ps_o_by_q[(b, qb)]
                pt = pts.pop(g)
                for h in range(HPC):
                    s = 32 * b + 2 * j + h
                    nc.tensor.matmul(
                        ps_o[h][:],
                        lhsT=vnat[:, s, :],
                        rhs=pt[:, QB * h:QB * (h + 1)],
                        start=(j == 0), stop=(j == NKV - 1),
                        skip_group_check=True)
                if j == NKV - 1:
                    normalize(b, qb)

            def normalize(b, qb):
                # drain ps_o to SBUF with two proven-shape copies (64 and
                # 1 partitions) so the 2 accumulator banks recycle fast;
                # recip + DRAM broadcast roundtrip + mul run off the
                # critical path on SBUF data
                c0 = 2048 * b + QB * qb
                ps_o = ps_o_by_q.pop((b, qb))
                for h in range(HPC):
                    osum = lbc_pool.tile([64, QB], f32, tag="osum",
                                         name="osum")
                    nc.vector.tensor_copy(out=osum[:], in_=ps_o[h][0:64, :])
                    lsb = lbc_pool.tile([1, QB], f32, tag="lsb", name="lsb")
                    nc.vector.tensor_copy(out=lsb[:], in_=ps_o[h][64:65, :])
                    lrec = lbc_pool.tile([1, QB], f32, tag="lrec",
                                         name="lrec")
                    nc.vector.reciprocal_approx_fast(
                        out=lrec[:], in_=lsb[:])
                    ldram = dram.tile([1, QB], f32, tag="ld", name="ld")
                    nc.sync.dma_start(out=ldram[:], in_=lrec[:])
                    lbc = lbc_pool.tile([64, QB], f32, tag="lbc", name="lbc")
                    nc.sync.dma_start(out=lbc[:],
                                      in_=ldram[:].to_broadcast([64, QB]))
                    nc.vector.tensor_mul(
                        out=ot[64 * h:64 * (h + 1), c0:c0 + QB],
                        in0=osum[:], in1=lbc[:])
                a2a_stage(b, qb)

            # ---- A2A half (reshard head-space -> row-space) ----
            a2a_outs = {}
            a2a_ins = {}

            def a2a_stage(half, qb):
                if half not in a2a_ins:
                    a2a_ins[half] = dram.tile([NCORES, 128, RHALF], bf16,
                                              tag="a2ain", name="a2ain")
                a_in = a2a_ins[half]
                for j in (2 * qb, 2 * qb + 1):
                    nc.sync.dma_start(
                        out=a_in[j],
                        in_=ot[:, 2048 * half + RHALF * j:
                               2048 * half + RHALF * (j + 1)])

            def a2a_half(half):
                from concourse import mybir as _mb
                a_in = a2a_ins[half]
                a_out = dram.tile([NCORES, 128, RHALF], bf16, tag="a2aout",
                                  name="a2aout")
                nc.gpsimd.collective_compute(
                    "AllToAll", _mb.AluOpType.bypass, replica_groups=rg,
                    ins=[a_in.opt()], outs=[a_out.opt()])
                a2a_outs[half] = a_out

            pins_by_half = {}

            def pins_load(half):
                # on the gpsimd queue so the wait-for-collective does not
                # head-of-line-block the sync queue's normalize DMAs
                a_out = a2a_outs[half]
                pins = []
                for j in range(NCORES):
                    p = pin_pool.tile([128, RHALF], bf16, tag="pin",
                                      name="pin")
                    nc.gpsimd.dma_start(out=p[:], in_=a_out[j])
                    pins.append(p)
                pins_by_half[half] = pins

            proj_state = {}

            def proj_unit(half, rb, n):
                pins = pins_by_half[half]
                if (half, rb) not in proj_state:
                    proj_state[(half, rb)] = (
                        out_pool.tile([128, D], f32, tag="osb", name="osb"),
                        hot.tile([128, 1024], f32, tag="hot", name="pjps"))
                osb, ps = proj_state[(half, rb)]
                for j in range(NCORES):
                    nc.tensor.matmul(
                        ps[:, 512 * n:512 * (n + 1)],
                        lhsT=pins[j][:, 128 * rb:128 * (rb + 1)],
                        rhs=wproj_sb[:, j, 512 * n:512 * (n + 1)],
                        start=(j == 0),
                        stop=(j == NCORES - 1) and not with_bproj)
                if with_bproj:
                    nc.tensor.matmul(
                        ps[:, 512 * n:512 * (n + 1)], lhsT=ones_col[:],
                        rhs=bproj_sb[:, 512 * n:512 * (n + 1)],
                        start=False, stop=True)
                nc.vector.tensor_copy(out=osb[:, 512 * n:512 * (n + 1)],
                                      in_=ps[:, 512 * n:512 * (n + 1)])
                if n == 1:
                    del proj_state[(half, rb)]
                    r0 = 256 * half + 128 * rb
                    nc.sync.dma_start(out=out[r0:r0 + 128, :], in_=osb[:])

            def proj_rb(half, rb):
                proj_unit(half, rb, 0)
                proj_unit(half, rb, 1)

            # ---- emission order ----
            # u[tb] = [dma, qk0a, qk0b, qk1a, qk1b, v0, v1, v2, v3]
            u = {tb: qkv_tblock_units(tb) for tb in range(NTB)}
            for f in u[0][:5]:    # tb0 dma+qk only; v deferred so the
                f()               # first S^T isn't queued behind it
            fill = {}

            def put(g, *fns):
                fill.setdefault(g, []).extend(fns)

            # b0 phase: just-in-time qk for tb1-3 (kt chunk 4j needed by
            # chunk 4j), V units placed PVLAG chunks before their PV
            put(0, u[0][5])
            put(1, u[1][0], u[1][1])
            put(2, u[1][2], u[1][3])
            put(3, u[1][4], u[0][6])
            put(4, u[0][7], u[0][8])
            put(5, u[2][0], u[1][5])
            put(6, u[2][1], u[2][2])
            put(7, u[2][3], u[2][4])
            put(8, u[1][6], u[1][7])
            put(9, u[3][0], u[1][8])
            put(10, u[3][1], u[3][2])
            put(11, u[3][3], u[3][4])
            put(12, u[2][5], u[2][6])
            put(13, u[2][7], u[2][8])
            put(14, u[3][5], u[3][6])
            put(15, u[3][7], u[3][8])
            # b1 QKV: no urgency, one unit per chunk
            for i, tb in enumerate((4, 5, 6, 7)):
                g0 = 18 + 10 * i
                for k in range(9):
                    put(g0 + k, u[tb][k])
            put(57, emit_wproj_loads)
            put(76, lambda: a2a_half(0))
            put(82, lambda: pins_load(0))

            NG = len(CHUNKS)
            for g in range(NG + PVLAG):
                if g in fill:
                    for fn in fill[g]:
                        fn()
                if g < NG:
                    emit_st(g)
                if g >= PVLAG:
                    emit_pv(g - PVLAG)
            # final reshard + projection; proj(0) runs inside the second
            # collective's window, with dummy matmuls sprinkled in to keep
            # the PE's HAM clock warm until proj(1)'s inputs arrive
            def dummy_mms(n):
                ps = hot.tile([128, 1024], f32, tag="hot", name="warm")
                for i in range(n):
                    nc.tensor.matmul(
                        ps[:, 512 * (i % 2):512 * (i % 2 + 1)],
                        lhsT=wproj_sb[:, 0, 0:128],
                        rhs=wproj_sb[:, 1, 0:512],
                        start=True, stop=True)

            a2a_half(1)
            dummy_mms(4)
            proj_unit(0, 0, 0)
            dummy_mms(2)
            proj_unit(0, 0, 1)
            dummy_mms(2)
            proj_unit(0, 1, 0)
            dummy_mms(2)
            proj_unit(0, 1, 1)
            dummy_mms(24)
            pins_load(1)
            proj_rb(1, 0)
            proj_rb(1, 1)

    nc.finalize()
    return nc


def _prep_inputs(x, attention_mask, w_attn, b_attn, w_proj, b_proj):
    x = np.asarray(x, np.float32)
    xt = np.ascontiguousarray(x.reshape(BT, D).T).astype(BF16)
    w_attn = np.asarray(w_attn, np.float32)
    b_attn = np.asarray(b_attn, np.float32)
    wp = np.ascontiguousarray(np.asarray(w_proj, np.float32)).astype(BF16)
    scale = 1.0 / np.sqrt(HD)
    am = np.asarray(attention_mask, np.float32)
    with_mask = bool(np.any(am))
    with_battn = bool(np.any(b_attn))
    with_bproj = bool(np.any(np.asarray(b_proj)))
    mask2 = None
    if with_mask:
        m2 = (2.0 * am).reshape(B, T // 128, 128)
        mask2 = np.ascontiguousarray(m2.transpose(2, 0, 1).reshape(128, -1))
    in_maps = []
    for c in range(NCORES):
        h0, h1 = HPC * c, HPC * c + 1
        qkcols = []
        vcols = []
        for h in (h0, h1):
            qkcols.append(w_attn[:, HD * h:HD * (h + 1)] * scale)      # q
            qkcols.append(w_attn[:, D + HD * h:D + HD * (h + 1)])      # k
            vcols.append(w_attn[:, 2 * D + HD * h:2 * D + HD * (h + 1)])
        wqk = np.ascontiguousarray(np.concatenate(qkcols, axis=1)).astype(BF16)
        wv = np.ascontiguousarray(np.concatenate(vcols, axis=1)).astype(BF16)
        m = {"xt": xt, "w_qk": wqk, "w_v": wv, "w_proj": wp}
        if with_mask:
            m["mask2"] = mask2
        if with_battn:
            bqk = []
            bv = []
            for h in (h0, h1):
                bqk.append(b_attn[HD * h:HD * (h + 1)] * scale)
                bqk.append(b_attn[D + HD * h:D + HD * (h + 1)])
                bv.append(b_attn[2 * D + HD * h:2 * D + HD * (h + 1)])
            m["b_qk"] = np.ascontiguousarray(
                np.concatenate(bqk)[None, :].astype(BF16))
            m["b_v"] = np.ascontiguousarray(
                np.concatenate(bv)[None, :].astype(BF16))
        if with_bproj:
            m["b_proj"] = np.ascontiguousarray(
                np.asarray(b_proj, np.float32)[None, :].astype(BF16))
        in_maps.append(m)
    return in_maps, (with_mask, with_battn, with_bproj)


def _run(inputs, trace=False, tmpdir=None):
    from concourse.bass_utils import run_bass_kernel_spmd

    in_maps, key = _prep_inputs(**inputs)
    if key not in _CACHE:
        _CACHE[key] = _build(*key)
    nc = _CACHE[key]
    try:
        res = run_bass_kernel_spmd(nc, in_maps, core_ids=list(range(NCORES)),
                                   trace=trace, tmpdir=tmpdir)
    except Exception as e:
        if "unrecoverable" not in str(e) and "UNAVAILABLE" not in str(e):
            raise
        import ctypes
        lib = ctypes.CDLL("/opt/axon/libaxon_pjrt.so")
        if hasattr(lib, "axon_reset"):
            lib.axon_reset.restype = ctypes.c_int64
            lib.axon_reset()
        res = run_bass_kernel_spmd(nc, in_maps, core_ids=list(range(NCORES)),
                                   trace=trace, tmpdir=tmpdir)
    y = np.empty((B, T, D), np.float32)
    for c in range(NCORES):
        o = res.results[c]["out"]
        y[0, RHALF * c:RHALF * (c + 1)] = o[:RHALF]
        y[1, RHALF * c:RHALF * (c + 1)] = o[RHALF:]
    return y, res


def kernel(**inputs) -> np.ndarray:
    y, _ = _run(inputs, trace=False)
    return y
